# revision 24
# baseline (speedup 1.0000x reference)
"""Trainium2 Bass kernel for nn_Decoder_46042049413334.

Buggy 2-layer LSTM decoder with attention (B=32, T=64, S=128, D=512).

Structure (per core, batch sharded 8 ways, BS=4 examples/core):
  Phase A: xpart0 = [emb(tokens), 1] @ [W_ih0.T; b0]  -> XPsb0 (SBUF)
  Interleaved pass: layer-0 step t and layer-1 step t-2 run together;
    layer-1's xpart is accumulated per step directly into its gates
    PSUM from the transposed h2 history (hT0), so the PE stays busy
    enough to hold the HAM clock gate open (K=8/8).
  Phase E: attention + out-projection from hT1

Recurrence layout: gates PSUM [128, 512] where partition 32*j+b holds
(example b, d-block j) and the 512 free cols are {i,f,o,2g}x128 for
that d-block (g columns pre-scaled by 2 so tanh(g) = 2*sigmoid(2g)-1
comes out of a single full-width sigmoid). The four d-blocks' weight
streams run CONCURRENTLY in the PE array via tile_position=(0, 32*j)
column tiling. Elementwise runs once over all 128 partitions; c2 and
h2 are re-transposed per step ([128,128] PE transpose). Off-chain work
(tanh(c2), h2, its transpose/gather) is emitted one step late so no
engine FIFO ever blocks the recurrence chains.

Row ordering is b-major everywhere: row r = b_local*T + t.
"""
import numpy as np
import ml_dtypes
from contextlib import ExitStack

import concourse.bass as bass
import concourse.bacc as bacc
import concourse.tile as tile
from concourse import mybir, masks
from concourse.bass_utils import run_bass_kernel_spmd

F32 = mybir.dt.float32
BF16 = mybir.dt.bfloat16
AF = mybir.ActivationFunctionType
NPBF = ml_dtypes.bfloat16

B, T, S, D, L, V = 32, 64, 128, 512, 2, 32000
G = 4 * D        # 2048
DS = 2 * D       # 1024
NCORES = 8
BS = B // NCORES  # 4
R = BS * T        # 256 rows per core
LAG = 2          # layer-1 recurrence lag behind layer 0


# ---------------------------------------------------------------- host side

def _gate_perm():
    perm = np.zeros(G, dtype=np.int64)
    base = {0: 0, 1: 512, 2: 1536, 3: 1024}  # i, f, o, g
    for j in range(G):
        nb, pos = divmod(j, 512)
        sub, dd = divmod(pos, 128)
        perm[j] = base[sub] + nb * 128 + dd
    return perm


def host_prep(inputs):
    """Build the 8 per-core input maps (layout/gather work only)."""
    perm = _gate_perm()
    tokens = np.asarray(inputs["prev_tgt_tokens"])
    embed = np.asarray(inputs["embed"], dtype=np.float32)
    enc = np.asarray(inputs["encoder_out"], dtype=np.float32)
    mask = np.asarray(inputs["src_mask"])
    hid = np.asarray(inputs["hiddens"], dtype=np.float32)
    cells = np.asarray(inputs["cells"], dtype=np.float32)
    W_ih = np.asarray(inputs["W_ih"], dtype=np.float32)
    W_hh = np.asarray(inputs["W_hh"], dtype=np.float32)
    b_ih = np.asarray(inputs["b_ih"], dtype=np.float32)
    b_hh = np.asarray(inputs["b_hh"], dtype=np.float32)
    W_in = np.asarray(inputs["W_in"], dtype=np.float32)
    b_in = np.asarray(inputs["b_in"], dtype=np.float32)
    W_out = np.asarray(inputs["W_out"], dtype=np.float32)
    b_out = np.asarray(inputs["b_out"], dtype=np.float32)

    def bf(x):
        return np.ascontiguousarray(x, dtype=NPBF)

    WIH = []
    WHH = []
    gscale = np.ones(G, np.float32)
    for nb in range(4):
        gscale[512 * nb + 384:512 * (nb + 1)] = 2.0   # tanh(g)=2*sig(2g)-1
    for l in range(L):
        wihT = W_ih[l].T[:, perm] * gscale
        biasrow = ((b_ih[l] + b_hh[l])[perm] * gscale)[None, :]
        WIH.append(bf(np.concatenate([wihT, biasrow], 0)))   # [513, 2048]
        WHH.append(bf(W_hh[l].T[:, perm] * gscale))          # [512, 2048]
    WINT = bf(W_in.T)                                        # [512, 1024]
    WOUTT = bf(np.concatenate([W_out.T, b_out[None, :]], 0))  # [1537, 512]

    # xpart0 injection selectors, one per u = t%8:
    # XPsb0 partition (4j+b)*8+u feeds gates row 32j+b
    einj8 = np.zeros((8, 128, 128), np.float32)
    for u in range(8):
        for j in range(4):
            for b in range(BS):
                einj8[u, (4 * j + b) * 8 + u, 32 * j + b] = 1.0
    einj8 = bf(einj8.transpose(1, 0, 2).reshape(128, 8 * 128))

    # block selector: e4blk[j, p] = 1 iff p // 32 == j
    e4 = np.zeros((4, 128), np.float32)
    for j in range(4):
        e4[j, 32 * j:32 * (j + 1)] = 1.0
    e4 = bf(e4)

    in_maps = []
    for core in range(NCORES):
        bsl = slice(core * BS, (core + 1) * BS)
        xe = embed[tokens[bsl]]                              # [BS, T, D]
        Xaug = np.concatenate(
            [xe.reshape(R, D), np.ones((R, 1), np.float32)], axis=1)
        # permute rows so phase-A store DMAs land partition-parallel:
        # new row b*64 + (t%8)*8 + t//8  <- (b, t)
        rperm = np.zeros(R, np.int64)
        for b in range(BS):
            for t in range(T):
                rperm[b * T + (t % 8) * 8 + t // 8] = b * T + t
        XT0 = bf(Xaug[rperm].T)                              # [513, 256]
        enc_c = np.ascontiguousarray(enc[bsl])               # [BS, 128, 1024]
        encT_c = np.swapaxes(enc_c, 1, 2)                    # [BS, 1024, 128]
        offs = np.einsum("bsd,d->bs", enc_c, b_in) + np.where(mask[bsl], -1e9, 0.0)
        offs_rep = np.ascontiguousarray(
            np.broadcast_to(offs[:, None, :], (BS, T, S)), dtype=np.float32)
        # initial c2T: c2t0[l, p, 32k+b] = hid[l, b, 128k+p]
        th = hid[:, bsl].reshape(L, BS, 4, 128).transpose(0, 3, 2, 1)  # [L,128,4,BS]
        c2t0 = np.zeros((L, 128, 4, 32), np.float32)
        c2t0[:, :, :, 0:BS] = th
        c2t0 = bf(c2t0.reshape(L, 128, 128))
        # cells in partition layout: cellsp[l, 32j+b, p] = cells[l, b, 128j+p]
        tc_ = cells[:, bsl].reshape(L, BS, 4, 128).transpose(0, 2, 1, 3)  # [L,4,BS,128]
        cellsp = np.zeros((L, 4, 32, 128), np.float32)
        cellsp[:, :, 0:BS, :] = tc_
        cellsp = bf(cellsp.reshape(L, 128, 128))
        in_maps.append({
            "xt0": XT0,
            "wih0": WIH[0], "whh0": WHH[0],
            "wih1": WIH[1], "whh1": WHH[1],
            "wint": WINT, "woutt": WOUTT,
            "enc": bf(enc_c), "enct": bf(encT_c), "offs": offs_rep,
            "c2t0": c2t0, "cellsp": cellsp,
            "ones1": np.ones((1, R), NPBF),
            "einj8": einj8, "e4blk": e4,
        })
    return in_maps


# ------------------------------------------------------------- device build

def build_program():
    nc = bacc.Bacc("TRN2", target_bir_lowering=False, debug=False)

    XT0 = nc.dram_tensor("xt0", [513, R], BF16, kind="ExternalInput")
    WIH0 = nc.dram_tensor("wih0", [513, G], BF16, kind="ExternalInput")
    WHH0 = nc.dram_tensor("whh0", [D, G], BF16, kind="ExternalInput")
    WIH1 = nc.dram_tensor("wih1", [513, G], BF16, kind="ExternalInput")
    WHH1 = nc.dram_tensor("whh1", [D, G], BF16, kind="ExternalInput")
    WINT = nc.dram_tensor("wint", [D, DS], BF16, kind="ExternalInput")
    WOUTT = nc.dram_tensor("woutt", [DS + D + 1, D], BF16, kind="ExternalInput")
    ENC = nc.dram_tensor("enc", [BS, S, DS], BF16, kind="ExternalInput")
    ENCT = nc.dram_tensor("enct", [BS, DS, S], BF16, kind="ExternalInput")
    OFFS = nc.dram_tensor("offs", [BS, T, S], F32, kind="ExternalInput")
    C2T0 = nc.dram_tensor("c2t0", [L, 128, 128], BF16, kind="ExternalInput")
    CELLSP = nc.dram_tensor("cellsp", [L, 128, 128], BF16, kind="ExternalInput")
    ONES1 = nc.dram_tensor("ones1", [1, R], BF16, kind="ExternalInput")
    EINJ = nc.dram_tensor("einj8", [128, 8 * 128], BF16, kind="ExternalInput")
    E4BLK = nc.dram_tensor("e4blk", [4, 128], BF16, kind="ExternalInput")
    OUT = nc.dram_tensor("out", [BS, T, D], F32, kind="ExternalOutput")

    with tile.TileContext(nc) as tc, ExitStack() as ctx:
        cpool = ctx.enter_context(tc.tile_pool(name="const", bufs=1))
        ident = cpool.tile([128, 128], F32)
        masks.make_identity(nc, ident[:])
        identb = cpool.tile([128, 128], BF16, name="identb")
        masks.make_identity(nc, identb[:])
        ones = cpool.tile([1, R], BF16, name="ones")
        nc.sync.dma_start(ones[:], ONES1.ap())
        einj8 = cpool.tile([128, 8 * 128], BF16, name="einj8")
        nc.sync.dma_start(einj8[:], EINJ.ap())
        e4blk = cpool.tile([4, 128], BF16, name="e4blk")
        nc.sync.dma_start(e4blk[:], E4BLK.ap())
        ones128 = cpool.tile([128, 128], BF16, name="ones128")
        nc.gpsimd.memset(ones128[:], 1.0)

        psp = ctx.enter_context(tc.tile_pool(name="ps", bufs=1, space="PSUM"))

        def gtile(idx, shape):
            return psp.tile(shape, F32, tag=f"g{idx}", name=f"g{idx}",
                            bufs=2 if idx < 2 else 1)

        # persistent SBUF xpart0:
        # XPsb0[(4*nb+b)*8 + t%8, (t//8)*512 + c] = xpart0[b,t,512nb+c]
        xpp = ctx.enter_context(tc.tile_pool(name="xps", bufs=1))
        XPsb0 = xpp.tile([128, (T // 8) * 512], BF16, name="xpsb0")

        # transposed h2 history per layer: hT[p, k*256 + b*64 + t]
        hT = [xpp.tile([128, 4 * R], BF16, name=f"hT{l}") for l in range(L)]

        # ---------------- Phase A inputs (packed, few DMAs) ----------------
        pa = ctx.enter_context(tc.tile_pool(name="pa", bufs=1))
        xtt = pa.tile([128, 4 * R], BF16, tag="xtt", name="xtt")
        nc.sync.dma_start(
            xtt[:].rearrange("p (k c) -> p k c", k=4),
            XT0.ap()[0:512].rearrange("(k p) c -> p k c", k=4))
        xt4 = pa.tile([1, R], BF16, tag="xt4", name="xt4")
        nc.sync.dma_start(xt4[:], XT0.ap()[512:513, :])

        # PE warm-up: dummy matmuls on the identity while DMAs land
        wps = psp.tile([128, 128], F32, tag="g0", name="g0", bufs=2)
        for w in range(48):
            nc.tensor.matmul(wps[:], identb[:], identb[:],
                             start=True, stop=True, skip_group_check=True)

        # prefetch pool: recurrence weights + attention operands
        pf = ctx.enter_context(tc.tile_pool(name="pf", bufs=1))
        cTb = pf.tile([128, 2 * 128], BF16, tag="cTb", name="cTb")
        nc.sync.dma_start(
            cTb[:].rearrange("p (l c) -> p l c", l=2),
            CELLSP.ap().rearrange("l p c -> p l c"))
        c2T0b = pf.tile([128, 2 * 128], BF16, tag="c2T0b", name="c2T0b")
        nc.sync.dma_start(
            c2T0b[:].rearrange("p (l c) -> p l c", l=2),
            C2T0.ap().rearrange("l p c -> p l c"))
        wkt = pa.tile([128, 4 * G], BF16, tag="wkt", name="wkt")
        for k in range(4):
            nc.sync.dma_start(
                wkt[:, 2048 * k:2048 * (k + 1)],
                WIH0.ap()[128 * k:128 * (k + 1), :])
        wk4 = pa.tile([1, G], BF16, tag="wk4", name="wk4")
        nc.sync.dma_start(wk4[:], WIH0.ap()[512:513, :])
        whht = []
        for l, Wd in ((0, WHH0), (1, WHH1)):
            wt = pf.tile([128, 4 * G], BF16, tag=f"whh{l}", name=f"whh{l}")
            for k in range(4):
                nc.sync.dma_start(
                    wt[:, 2048 * k:2048 * (k + 1)],
                    Wd.ap()[128 * k:128 * (k + 1), :])
            whht.append(wt)
        wihC = pf.tile([128, 4 * G], BF16, tag="wihC", name="wihC")
        nc.sync.dma_start(
            wihC[:].rearrange("p (k c) -> p k c", k=4),
            WIH1.ap()[0:512].rearrange("(k p) c -> p k c", k=4))
        wihC4 = pf.tile([4, 512], BF16, tag="wihC4", name="wihC4")
        nc.sync.dma_start(wihC4[:], WIH1.ap()[512:513, :].rearrange(
            "a (j c) -> (a j) c", j=4))
        # ---------------- Phase A: xpart0 ----------------
        for mc in range(2):
            for nb in range(4):
                ps = gtile(nb % 2, [128, 512])
                for k in range(4):
                    nc.tensor.matmul(
                        ps[:],
                        xtt[:, 256 * k + 128 * mc:256 * k + 128 * (mc + 1)],
                        wkt[:, 2048 * k + 512 * nb:2048 * k + 512 * (nb + 1)],
                        start=(k == 0), stop=False)
                nc.tensor.matmul(
                    ps[:], xt4[:, 128 * mc:128 * (mc + 1)],
                    wk4[:, 512 * nb:512 * (nb + 1)],
                    start=False, stop=True)
                sb = pa.tile([128, 512], BF16, tag=f"stg{nb}", name=f"stg{nb}")
                nc.scalar.copy(sb[:], ps[:])
                p0 = (4 * nb + 2 * mc) * 8
                dst = XPsb0[p0:p0 + 16, :].rearrange(
                    "p (q c) -> p q c", c=512)
                nc.sync.dma_start(dst, sb[:])

        # ---------------- Interleaved recurrence passes ----------------
        rp = ctx.enter_context(tc.tile_pool(name="rp", bufs=2))

        def linit(l):
            return {"l": l, "cT": cTb[:, 128 * l:128 * (l + 1)],
                    "c2T": c2T0b[:, 128 * l:128 * (l + 1)], "whh": whht[l],
                    "sall_prev": None, "c2h_prev": None, "tprev": -1}

        def lstep_mm(st, t):
            """Inject/xpart + W_hh rounds for step t (PE bulk)."""
            l = st["l"]
            gates = gtile(l, [128, 512])
            if l == 0:
                rhs = XPsb0[:, 512 * (t // 8):512 * (t // 8 + 1)]
                nc.tensor.matmul(
                    gates[:], einj8[:, 128 * (t % 8):128 * (t % 8 + 1)], rhs,
                    start=True, stop=False, skip_group_check=True)
            else:
                # xpart1(t) accumulated in place: bias row first (writes all
                # 128 partitions), then W_ih1 rounds from hT0 columns of t
                nc.tensor.matmul(gates[:], e4blk[:], wihC4[:],
                                 start=True, stop=False, skip_group_check=True)
                for k in range(4):
                    lhsT = hT[0][:].rearrange(
                        "p (k b t) -> p k b t", k=4, b=BS)[:, k, :, t]
                    for j in range(4):
                        nc.tensor.matmul(
                            gates[32 * j:32 * j + BS, :],
                            lhsT,
                            wihC[:, 2048 * k + 512 * j:2048 * k + 512 * (j + 1)],
                            start=False, stop=False,
                            tile_position=(0, 32 * j), skip_group_check=True)
            for k in range(4):
                lhsT = st["c2T"][:, 32 * k:32 * k + BS]
                for j in range(4):
                    nc.tensor.matmul(
                        gates[32 * j:32 * j + BS, :],
                        lhsT,
                        st["whh"][:, 2048 * k + 512 * j:2048 * k + 512 * (j + 1)],
                        start=False, stop=(k == 3),
                        tile_position=(0, 32 * j), skip_group_check=True)
            st["gates"] = gates

        def lstep_sigma(st, t):
            """sigma + c2 elementwise chain for step t (no transpose)."""
            l = st["l"]
            gates = st["gates"]
            sall = rp.tile([128, 512], F32, tag=f"sa{l}", name=f"sa{l}",
                           bufs=3)
            nc.scalar.activation(sall[:], gates[:], AF.Sigmoid)
            m1 = rp.tile([128, 128], BF16, tag=f"m1{l}", name=f"m1{l}")
            nc.gpsimd.tensor_mul(m1[:], sall[:, 128:256], st["cT"])
            tgv = rp.tile([128, 128], BF16, tag=f"tg{l}", name=f"tg{l}")
            nc.vector.scalar_tensor_tensor(
                tgv[:], sall[:, 384:512], 2.0, ones128[:],
                mybir.AluOpType.mult, mybir.AluOpType.subtract)
            m2 = rp.tile([128, 128], BF16, tag=f"m2{l}", name=f"m2{l}")
            nc.vector.tensor_mul(m2[:], sall[:, 0:128], tgv[:])
            c2h = rp.tile([128, 128], BF16, tag=f"c2h{l}", name=f"c2h{l}",
                          bufs=3)
            nc.vector.tensor_add(c2h[:], m1[:], m2[:])
            st["sall"] = sall
            st["c2h"] = c2h

        def lstep_transpose(st):
            """c2 transpose + cast — emitted at a PE-queue position where
            the chain dependency (add) has already resolved."""
            l = st["l"]
            tp = psp.tile([128, 128], BF16, tag=f"tp{l}", name=f"tp{l}",
                          bufs=2)
            nc.tensor.transpose(tp[:], st["c2h"][:], identb[:])
            c2T_new = rp.tile([128, 128], BF16, tag=f"c2T{l}", name=f"c2T{l}")
            nc.vector.tensor_copy(c2T_new[:], tp[:])
            st["c2T"] = c2T_new

        def lstep_branch(st):
            """Delayed h2 branch for the PREVIOUS step (never blocks chains)."""
            l = st["l"]
            if st["sall_prev"] is not None:
                tc2 = rp.tile([128, 128], BF16, tag=f"tc2{l}", name=f"tc2{l}")
                nc.scalar.activation(tc2[:], st["c2h_prev"][:], AF.Tanh)
                h2 = rp.tile([128, 128], BF16, tag=f"h2{l}", name=f"h2{l}")
                nc.gpsimd.tensor_mul(h2[:], st["sall_prev"][:, 256:384], tc2[:])
                tp2 = psp.tile([128, 128], BF16, tag=f"tp{l}", name=f"tp{l}",
                               bufs=2)
                nc.tensor.transpose(tp2[:], h2[:], identb[:])
                src_ = tp2[:].rearrange("p (k r) -> p k r", k=4)[:, :, 0:BS]
                dst = hT[l][:].rearrange(
                    "p (k b t) -> p k b t", k=4, b=BS)[:, :, :, st["tprev"]]
                nc.vector.tensor_copy(dst, src_)
            st["sall_prev"] = st["sall"]
            st["c2h_prev"] = st["c2h"]
            st["tprev"] = st["tprev"] + 1

        # phase-E operands: transferred during the recurrence pass
        # phase-E operands: transferred during the recurrence pass
        wint = pf.tile([128, 4 * DS], BF16, tag="wint", name="wint")
        nc.sync.dma_start(
            wint[:].rearrange("p (k c) -> p k c", k=4),
            WINT.ap().rearrange("(k p) c -> p k c", k=4))
        encb = pf.tile([S, 4 * DS], BF16, tag="encb", name="encb")
        nc.sync.dma_start(
            encb[:].rearrange("p (b c) -> p b c", b=BS),
            ENC.ap().rearrange("b s d -> s b d"))
        enctb = pf.tile([128, BS * 8 * S], BF16, tag="enctb", name="enctb")
        nc.sync.dma_start(
            enctb[:].rearrange("p (b k s) -> p b k s", b=BS, k=8),
            ENCT.ap().rearrange("b (k p) s -> p b k s", k=8))
        offsb = pf.tile([T, BS * S], F32, tag="offsb", name="offsb")
        nc.sync.dma_start(
            offsb[:].rearrange("p (b s) -> p b s", b=BS),
            OFFS.ap().rearrange("b t s -> t b s"))
        woutt = pf.tile([128, 12 * D], BF16, tag="woutt", name="woutt")
        nc.sync.dma_start(
            woutt[:].rearrange("p (k c) -> p k c", k=12),
            WOUTT.ap()[0:1536].rearrange("(k p) c -> p k c", k=12))
        woutb = pf.tile([1, D], BF16, tag="woutb", name="woutb")
        nc.sync.dma_start(woutb[:], WOUTT.ap()[1536:1537, :])

        st0 = linit(0)
        st1 = linit(1)
        for ss in range(T + LAG + 1):
            if ss < T:
                lstep_mm(st0, ss)                # PE: inj + rounds L0(t)
            if LAG < ss <= T + LAG:
                lstep_transpose(st1)             # PE: T_c2 L1(t'-1) (ready)
            if ss < T:
                lstep_sigma(st0, ss)
            if LAG <= ss < T + LAG:
                lstep_mm(st1, ss - LAG)          # PE: bias/xpart/whh L1(t')
            if LAG < ss <= T + LAG:
                lstep_branch(st1)                # PE: T_h2 L1(t'-1)
            if LAG <= ss < T + LAG:
                lstep_sigma(st1, ss - LAG)
            if ss < T:
                lstep_transpose(st0)             # PE: T_c2 L0(t) (ready)
            if ss <= T:
                lstep_branch(st0)                # PE: T_h2 L0(t-1)
        lstep_branch(st1)                        # flush T_h2 L1(63)

        # ---------------- Phase E: attention + out proj ----------------
        wkt2 = pa.tile([128, 4 * G], BF16, tag="wkt", name="wkt2")
        with tc.tile_pool(name="pe", bufs=1) as pe:
            sT = [hT[1][:, 256 * k:256 * (k + 1)] for k in range(4)]

            xqT = []
            for m in range(8):
                ps = gtile(m % 2, [128, R])
                for k in range(4):
                    nc.tensor.matmul(
                        ps[:], wint[:, 1024 * k + 128 * m:1024 * k + 128 * (m + 1)],
                        sT[k], start=(k == 0), stop=(k == 3))
                xq = wkt2[:, 256 * m:256 * (m + 1)]
                if m % 2 == 0:
                    nc.scalar.copy(xq, ps[:])
                else:
                    nc.vector.tensor_copy(xq, ps[:])
                xqT.append(xq)

            ctxT = [wkt2[:, 2048 + 256 * m:2048 + 256 * (m + 1)]
                    for m in range(8)]
            for b in range(BS):
                bsl = slice(T * b, T * (b + 1))
                eps = psp.tile([T, S], F32, tag=f"g{b % 2}",
                               name=f"g{b % 2}", bufs=2)
                for k in range(8):
                    nc.tensor.matmul(
                        eps[:], xqT[k][:, bsl],
                        enctb[:, 1024 * b + 128 * k:1024 * b + 128 * (k + 1)],
                        start=(k == 0), stop=(k == 7))
                esb = pe.tile([T, S], F32, tag=f"esb{b % 2}",
                              name=f"esb{b % 2}")
                nc.vector.tensor_add(esb[:], eps[:],
                                     offsb[:, 128 * b:128 * (b + 1)])
                negmax = pe.tile([T, 1], F32, tag=f"negmax{b % 2}",
                                 name=f"negmax{b % 2}")
                nc.vector.reduce_max(
                    negmax[:], esb[:], axis=mybir.AxisListType.X, negate=True)
                expE = pe.tile([T, S], F32, tag=f"expE{b % 2}",
                               name=f"expE{b % 2}")
                den = pe.tile([T, 1], F32, tag=f"den{b % 2}",
                              name=f"den{b % 2}")
                nc.scalar.activation(
                    expE[:], esb[:], AF.Exp, bias=negmax[:], accum_out=den[:])
                rden = pe.tile([T, 1], F32, tag=f"rden{b % 2}",
                               name=f"rden{b % 2}")
                nc.vector.reciprocal(rden[:], den[:])
                attn = pe.tile([T, S], F32, tag=f"attn{b % 2}",
                               name=f"attn{b % 2}")
                nc.vector.tensor_scalar_mul(attn[:], expE[:], rden[:])
                tpa = psp.tile([S, T], F32, tag=f"tp{b % 2}",
                               name=f"tp{b % 2}", bufs=2)
                nc.tensor.transpose(tpa[:], attn[:], ident[0:T, 0:T])
                atsb = pe.tile([S, T], BF16, tag=f"atsb{b % 2}",
                               name=f"atsb{b % 2}")
                nc.vector.tensor_copy(atsb[:], tpa[:])
                for m in range(8):
                    psc = gtile(m % 2, [128, T])
                    nc.tensor.matmul(
                        psc[:], encb[:, 1024 * b + 128 * m:1024 * b + 128 * (m + 1)],
                        atsb[:], start=True, stop=True)
                    if m % 2 == 0:
                        nc.scalar.copy(ctxT[m][:, bsl], psc[:])
                    else:
                        nc.vector.tensor_copy(ctxT[m][:, bsl], psc[:])

            outflat = OUT.ap().rearrange("b t d -> (b t) d")
            lhs_all = ctxT + sT + [ones]
            wt_all = [woutt[:, 512 * k:512 * (k + 1)] for k in range(12)] \
                + [woutb[:]]
            for mc in range(2):
                msl = slice(128 * mc, 128 * (mc + 1))
                ps = gtile(mc, [128, D])
                for k in range(13):
                    nc.tensor.matmul(
                        ps[:], lhs_all[k][:, msl], wt_all[k],
                        start=(k == 0), stop=(k == 12))
                osb = pa.tile([128, D], F32, tag=f"stg{mc}", name=f"osb{mc}")
                nc.scalar.activation(osb[:], ps[:], AF.Tanh)
                nc.sync.dma_start(outflat[msl, :], osb[:])

    nc.compile()
    return nc


def assemble(results):
    full = np.concatenate([r["out"] for r in results], axis=0)  # [B, T, D]
    outs = full.transpose(1, 0, 2)                              # [T, B, D]
    return np.ascontiguousarray(outs.reshape(-1, D).reshape(-1, T, D))


_nc_cache = None


def kernel(**inputs):
    global _nc_cache
    in_maps = host_prep(inputs)
    if _nc_cache is None:
        _nc_cache = build_program()
    res = run_bass_kernel_spmd(_nc_cache, in_maps, list(range(NCORES)))
    return assemble(res.results)


# revision 25
# speedup vs baseline: 1.0036x; 1.0036x over previous
"""Trainium2 Bass kernel for nn_Decoder_46042049413334.

Buggy 2-layer LSTM decoder with attention (B=32, T=64, S=128, D=512).

Structure (per core, batch sharded 8 ways, BS=4 examples/core):
  Phase A: xpart0 = [emb(tokens), 1] @ [W_ih0.T; b0]  -> XPsb0 (SBUF)
  Interleaved pass: layer-0 step t and layer-1 step t-2 run together;
    layer-1's xpart is accumulated per step directly into its gates
    PSUM from the transposed h2 history (hT0), so the PE stays busy
    enough to hold the HAM clock gate open (K=8/8).
  Phase E: attention + out-projection from hT1

Recurrence layout: gates PSUM [128, 512] where partition 32*j+b holds
(example b, d-block j) and the 512 free cols are {i,f,o,2g}x128 for
that d-block (g columns pre-scaled by 2 so tanh(g) = 2*sigmoid(2g)-1
comes out of a single full-width sigmoid). The four d-blocks' weight
streams run CONCURRENTLY in the PE array via tile_position=(0, 32*j)
column tiling. Elementwise runs once over all 128 partitions; c2 and
h2 are re-transposed per step ([128,128] PE transpose). Off-chain work
(tanh(c2), h2, its transpose/gather) is emitted one step late so no
engine FIFO ever blocks the recurrence chains.

Row ordering is b-major everywhere: row r = b_local*T + t.
"""
import numpy as np
import ml_dtypes
from contextlib import ExitStack

import concourse.bass as bass
import concourse.bacc as bacc
import concourse.tile as tile
from concourse import mybir, masks
from concourse.bass_utils import run_bass_kernel_spmd

F32 = mybir.dt.float32
BF16 = mybir.dt.bfloat16
AF = mybir.ActivationFunctionType
NPBF = ml_dtypes.bfloat16

B, T, S, D, L, V = 32, 64, 128, 512, 2, 32000
G = 4 * D        # 2048
DS = 2 * D       # 1024
NCORES = 8
BS = B // NCORES  # 4
R = BS * T        # 256 rows per core
LAG = 2          # layer-1 recurrence lag behind layer 0


# ---------------------------------------------------------------- host side

def _gate_perm():
    perm = np.zeros(G, dtype=np.int64)
    base = {0: 0, 1: 512, 2: 1536, 3: 1024}  # i, f, o, g
    for j in range(G):
        nb, pos = divmod(j, 512)
        sub, dd = divmod(pos, 128)
        perm[j] = base[sub] + nb * 128 + dd
    return perm


def host_prep(inputs):
    """Build the 8 per-core input maps (layout/gather work only)."""
    perm = _gate_perm()
    tokens = np.asarray(inputs["prev_tgt_tokens"])
    embed = np.asarray(inputs["embed"], dtype=np.float32)
    enc = np.asarray(inputs["encoder_out"], dtype=np.float32)
    mask = np.asarray(inputs["src_mask"])
    hid = np.asarray(inputs["hiddens"], dtype=np.float32)
    cells = np.asarray(inputs["cells"], dtype=np.float32)
    W_ih = np.asarray(inputs["W_ih"], dtype=np.float32)
    W_hh = np.asarray(inputs["W_hh"], dtype=np.float32)
    b_ih = np.asarray(inputs["b_ih"], dtype=np.float32)
    b_hh = np.asarray(inputs["b_hh"], dtype=np.float32)
    W_in = np.asarray(inputs["W_in"], dtype=np.float32)
    b_in = np.asarray(inputs["b_in"], dtype=np.float32)
    W_out = np.asarray(inputs["W_out"], dtype=np.float32)
    b_out = np.asarray(inputs["b_out"], dtype=np.float32)

    def bf(x):
        return np.ascontiguousarray(x, dtype=NPBF)

    WIH = []
    WHH = []
    gscale = np.ones(G, np.float32)
    for nb in range(4):
        gscale[512 * nb + 384:512 * (nb + 1)] = 2.0   # tanh(g)=2*sig(2g)-1
    for l in range(L):
        wihT = W_ih[l].T[:, perm] * gscale
        biasrow = ((b_ih[l] + b_hh[l])[perm] * gscale)[None, :]
        WIH.append(bf(np.concatenate([wihT, biasrow], 0)))   # [513, 2048]
        WHH.append(bf(W_hh[l].T[:, perm] * gscale))          # [512, 2048]
    WINT = bf(W_in.T)                                        # [512, 1024]
    WOUTT = bf(np.concatenate([W_out.T, b_out[None, :]], 0))  # [1537, 512]

    # xpart0 injection selectors, one per u = t%8:
    # XPsb0 partition (4j+b)*8+u feeds gates row 32j+b
    einj8 = np.zeros((8, 128, 128), np.float32)
    for u in range(8):
        for j in range(4):
            for b in range(BS):
                einj8[u, (4 * j + b) * 8 + u, 32 * j + b] = 1.0
    einj8 = bf(einj8.transpose(1, 0, 2).reshape(128, 8 * 128))

    # block selector: e4blk[j, p] = 1 iff p // 32 == j
    e4 = np.zeros((4, 128), np.float32)
    for j in range(4):
        e4[j, 32 * j:32 * (j + 1)] = 1.0
    e4 = bf(e4)

    in_maps = []
    for core in range(NCORES):
        bsl = slice(core * BS, (core + 1) * BS)
        xe = embed[tokens[bsl]]                              # [BS, T, D]
        Xaug = np.concatenate(
            [xe.reshape(R, D), np.ones((R, 1), np.float32)], axis=1)
        # permute rows so phase-A store DMAs land partition-parallel:
        # new row b*64 + (t%8)*8 + t//8  <- (b, t)
        rperm = np.zeros(R, np.int64)
        for b in range(BS):
            for t in range(T):
                rperm[b * T + (t % 8) * 8 + t // 8] = b * T + t
        XT0 = bf(Xaug[rperm].T)                              # [513, 256]
        enc_c = np.ascontiguousarray(enc[bsl])               # [BS, 128, 1024]
        encT_c = np.swapaxes(enc_c, 1, 2)                    # [BS, 1024, 128]
        offs = np.einsum("bsd,d->bs", enc_c, b_in) + np.where(mask[bsl], -1e9, 0.0)
        offs_rep = np.ascontiguousarray(
            np.broadcast_to(offs[:, None, :], (BS, T, S)), dtype=np.float32)
        # initial c2T: c2t0[l, p, 32k+b] = hid[l, b, 128k+p]
        th = hid[:, bsl].reshape(L, BS, 4, 128).transpose(0, 3, 2, 1)  # [L,128,4,BS]
        c2t0 = np.zeros((L, 128, 4, 32), np.float32)
        c2t0[:, :, :, 0:BS] = th
        c2t0 = bf(c2t0.reshape(L, 128, 128))
        # cells in partition layout: cellsp[l, 32j+b, p] = cells[l, b, 128j+p]
        tc_ = cells[:, bsl].reshape(L, BS, 4, 128).transpose(0, 2, 1, 3)  # [L,4,BS,128]
        cellsp = np.zeros((L, 4, 32, 128), np.float32)
        cellsp[:, :, 0:BS, :] = tc_
        cellsp = bf(cellsp.reshape(L, 128, 128))
        in_maps.append({
            "xt0": XT0,
            "wih0": WIH[0], "whh0": WHH[0],
            "wih1": WIH[1], "whh1": WHH[1],
            "wint": WINT, "woutt": WOUTT,
            "enc": bf(enc_c), "enct": bf(encT_c), "offs": offs_rep,
            "c2t0": c2t0, "cellsp": cellsp,
            "ones1": np.ones((1, R), NPBF),
            "einj8": einj8, "e4blk": e4,
        })
    return in_maps


# ------------------------------------------------------------- device build

def build_program():
    nc = bacc.Bacc("TRN2", target_bir_lowering=False, debug=False)

    XT0 = nc.dram_tensor("xt0", [513, R], BF16, kind="ExternalInput")
    WIH0 = nc.dram_tensor("wih0", [513, G], BF16, kind="ExternalInput")
    WHH0 = nc.dram_tensor("whh0", [D, G], BF16, kind="ExternalInput")
    WIH1 = nc.dram_tensor("wih1", [513, G], BF16, kind="ExternalInput")
    WHH1 = nc.dram_tensor("whh1", [D, G], BF16, kind="ExternalInput")
    WINT = nc.dram_tensor("wint", [D, DS], BF16, kind="ExternalInput")
    WOUTT = nc.dram_tensor("woutt", [DS + D + 1, D], BF16, kind="ExternalInput")
    ENC = nc.dram_tensor("enc", [BS, S, DS], BF16, kind="ExternalInput")
    ENCT = nc.dram_tensor("enct", [BS, DS, S], BF16, kind="ExternalInput")
    OFFS = nc.dram_tensor("offs", [BS, T, S], F32, kind="ExternalInput")
    C2T0 = nc.dram_tensor("c2t0", [L, 128, 128], BF16, kind="ExternalInput")
    CELLSP = nc.dram_tensor("cellsp", [L, 128, 128], BF16, kind="ExternalInput")
    ONES1 = nc.dram_tensor("ones1", [1, R], BF16, kind="ExternalInput")
    EINJ = nc.dram_tensor("einj8", [128, 8 * 128], BF16, kind="ExternalInput")
    E4BLK = nc.dram_tensor("e4blk", [4, 128], BF16, kind="ExternalInput")
    OUT = nc.dram_tensor("out", [BS, T, D], F32, kind="ExternalOutput")

    with tile.TileContext(nc) as tc, ExitStack() as ctx:
        cpool = ctx.enter_context(tc.tile_pool(name="const", bufs=1))
        ident = cpool.tile([128, 128], F32)
        masks.make_identity(nc, ident[:])
        identb = cpool.tile([128, 128], BF16, name="identb")
        masks.make_identity(nc, identb[:])
        ones = cpool.tile([1, R], BF16, name="ones")
        nc.sync.dma_start(ones[:], ONES1.ap())
        einj8 = cpool.tile([128, 8 * 128], BF16, name="einj8")
        nc.sync.dma_start(einj8[:], EINJ.ap())
        e4blk = cpool.tile([4, 128], BF16, name="e4blk")
        nc.sync.dma_start(e4blk[:], E4BLK.ap())
        ones128 = cpool.tile([128, 128], BF16, name="ones128")
        nc.gpsimd.memset(ones128[:], 1.0)
        tw = cpool.tile([1, 4], F32, name="tw")
        nc.scalar.activation(tw[:], ones128[0:1, 0:4], AF.Sigmoid)
        nc.scalar.activation(tw[:], ones128[0:1, 0:4], AF.Tanh)
        nc.scalar.activation(tw[:], ones128[0:1, 0:4], AF.Exp)

        psp = ctx.enter_context(tc.tile_pool(name="ps", bufs=1, space="PSUM"))

        def gtile(idx, shape):
            return psp.tile(shape, F32, tag=f"g{idx}", name=f"g{idx}",
                            bufs=2 if idx < 2 else 1)

        # persistent SBUF xpart0:
        # XPsb0[(4*nb+b)*8 + t%8, (t//8)*512 + c] = xpart0[b,t,512nb+c]
        xpp = ctx.enter_context(tc.tile_pool(name="xps", bufs=1))
        XPsb0 = xpp.tile([128, (T // 8) * 512], BF16, name="xpsb0")

        # transposed h2 history per layer: hT[p, k*256 + b*64 + t]
        hT = [xpp.tile([128, 4 * R], BF16, name=f"hT{l}") for l in range(L)]

        # ---------------- Phase A inputs (packed, few DMAs) ----------------
        pa = ctx.enter_context(tc.tile_pool(name="pa", bufs=1))
        xtt = pa.tile([128, 4 * R], BF16, tag="xtt", name="xtt")
        nc.sync.dma_start(
            xtt[:].rearrange("p (k c) -> p k c", k=4),
            XT0.ap()[0:512].rearrange("(k p) c -> p k c", k=4))
        xt4 = pa.tile([1, R], BF16, tag="xt4", name="xt4")
        nc.sync.dma_start(xt4[:], XT0.ap()[512:513, :])

        # PE warm-up: dummy matmuls on the identity while DMAs land
        wps = psp.tile([128, 128], F32, tag="g0", name="g0", bufs=2)
        for w in range(48):
            nc.tensor.matmul(wps[:], identb[:], identb[:],
                             start=True, stop=True, skip_group_check=True)

        # prefetch pool: recurrence weights + attention operands
        pf = ctx.enter_context(tc.tile_pool(name="pf", bufs=1))
        cTb = pf.tile([128, 2 * 128], BF16, tag="cTb", name="cTb")
        nc.sync.dma_start(
            cTb[:].rearrange("p (l c) -> p l c", l=2),
            CELLSP.ap().rearrange("l p c -> p l c"))
        c2T0b = pf.tile([128, 2 * 128], BF16, tag="c2T0b", name="c2T0b")
        nc.sync.dma_start(
            c2T0b[:].rearrange("p (l c) -> p l c", l=2),
            C2T0.ap().rearrange("l p c -> p l c"))
        wkt = pa.tile([128, 4 * G], BF16, tag="wkt", name="wkt")
        for k in range(4):
            nc.sync.dma_start(
                wkt[:, 2048 * k:2048 * (k + 1)],
                WIH0.ap()[128 * k:128 * (k + 1), :])
        wk4 = pa.tile([1, G], BF16, tag="wk4", name="wk4")
        nc.sync.dma_start(wk4[:], WIH0.ap()[512:513, :])
        whht = []
        for l, Wd in ((0, WHH0), (1, WHH1)):
            wt = pf.tile([128, 4 * G], BF16, tag=f"whh{l}", name=f"whh{l}")
            for k in range(4):
                nc.sync.dma_start(
                    wt[:, 2048 * k:2048 * (k + 1)],
                    Wd.ap()[128 * k:128 * (k + 1), :])
            whht.append(wt)
        wihC = pf.tile([128, 4 * G], BF16, tag="wihC", name="wihC")
        nc.sync.dma_start(
            wihC[:].rearrange("p (k c) -> p k c", k=4),
            WIH1.ap()[0:512].rearrange("(k p) c -> p k c", k=4))
        wihC4 = pf.tile([4, 512], BF16, tag="wihC4", name="wihC4")
        nc.sync.dma_start(wihC4[:], WIH1.ap()[512:513, :].rearrange(
            "a (j c) -> (a j) c", j=4))
        # ---------------- Phase A: xpart0 ----------------
        for mc in range(2):
            for nb in range(4):
                ps = gtile(nb % 2, [128, 512])
                for k in range(4):
                    nc.tensor.matmul(
                        ps[:],
                        xtt[:, 256 * k + 128 * mc:256 * k + 128 * (mc + 1)],
                        wkt[:, 2048 * k + 512 * nb:2048 * k + 512 * (nb + 1)],
                        start=(k == 0), stop=False)
                nc.tensor.matmul(
                    ps[:], xt4[:, 128 * mc:128 * (mc + 1)],
                    wk4[:, 512 * nb:512 * (nb + 1)],
                    start=False, stop=True)
                sb = pa.tile([128, 512], BF16, tag=f"stg{nb}", name=f"stg{nb}")
                nc.vector.tensor_copy(sb[:], ps[:])
                p0 = (4 * nb + 2 * mc) * 8
                dst = XPsb0[p0:p0 + 16, :].rearrange(
                    "p (q c) -> p q c", c=512)
                nc.sync.dma_start(dst, sb[:])

        # ---------------- Interleaved recurrence passes ----------------
        rp = ctx.enter_context(tc.tile_pool(name="rp", bufs=2))

        def linit(l):
            return {"l": l, "cT": cTb[:, 128 * l:128 * (l + 1)],
                    "c2T": c2T0b[:, 128 * l:128 * (l + 1)], "whh": whht[l],
                    "sall_prev": None, "c2h_prev": None, "tprev": -1}

        def lstep_mm(st, t):
            """Inject/xpart + W_hh rounds for step t (PE bulk)."""
            l = st["l"]
            gates = gtile(l, [128, 512])
            if l == 0:
                rhs = XPsb0[:, 512 * (t // 8):512 * (t // 8 + 1)]
                nc.tensor.matmul(
                    gates[:], einj8[:, 128 * (t % 8):128 * (t % 8 + 1)], rhs,
                    start=True, stop=False, skip_group_check=True)
            else:
                # xpart1(t) accumulated in place: bias row first (writes all
                # 128 partitions), then W_ih1 rounds from hT0 columns of t
                nc.tensor.matmul(gates[:], e4blk[:], wihC4[:],
                                 start=True, stop=False, skip_group_check=True)
                for k in range(4):
                    lhsT = hT[0][:].rearrange(
                        "p (k b t) -> p k b t", k=4, b=BS)[:, k, :, t]
                    for j in range(4):
                        nc.tensor.matmul(
                            gates[32 * j:32 * j + BS, :],
                            lhsT,
                            wihC[:, 2048 * k + 512 * j:2048 * k + 512 * (j + 1)],
                            start=False, stop=False,
                            tile_position=(0, 32 * j), skip_group_check=True)
            for k in range(4):
                lhsT = st["c2T"][:, 32 * k:32 * k + BS]
                for j in range(4):
                    nc.tensor.matmul(
                        gates[32 * j:32 * j + BS, :],
                        lhsT,
                        st["whh"][:, 2048 * k + 512 * j:2048 * k + 512 * (j + 1)],
                        start=False, stop=(k == 3),
                        tile_position=(0, 32 * j), skip_group_check=True)
            st["gates"] = gates

        def lstep_sigma(st, t):
            """sigma + c2 elementwise chain for step t (no transpose)."""
            l = st["l"]
            gates = st["gates"]
            sall = rp.tile([128, 512], F32, tag=f"sa{l}", name=f"sa{l}",
                           bufs=3)
            nc.scalar.activation(sall[:], gates[:], AF.Sigmoid)
            m1 = rp.tile([128, 128], BF16, tag=f"m1{l}", name=f"m1{l}")
            nc.gpsimd.tensor_mul(m1[:], sall[:, 128:256], st["cT"])
            tgv = rp.tile([128, 128], BF16, tag=f"tg{l}", name=f"tg{l}")
            nc.vector.scalar_tensor_tensor(
                tgv[:], sall[:, 384:512], 2.0, ones128[:],
                mybir.AluOpType.mult, mybir.AluOpType.subtract)
            m2 = rp.tile([128, 128], BF16, tag=f"m2{l}", name=f"m2{l}")
            nc.vector.tensor_mul(m2[:], sall[:, 0:128], tgv[:])
            c2h = rp.tile([128, 128], BF16, tag=f"c2h{l}", name=f"c2h{l}",
                          bufs=3)
            nc.vector.tensor_add(c2h[:], m1[:], m2[:])
            st["sall"] = sall
            st["c2h"] = c2h

        def lstep_transpose(st):
            """c2 transpose + cast — emitted at a PE-queue position where
            the chain dependency (add) has already resolved."""
            l = st["l"]
            tp = psp.tile([128, 128], BF16, tag=f"tp{l}", name=f"tp{l}",
                          bufs=2)
            nc.tensor.transpose(tp[:], st["c2h"][:], identb[:])
            c2T_new = rp.tile([128, 128], BF16, tag=f"c2T{l}", name=f"c2T{l}")
            nc.vector.tensor_copy(c2T_new[:], tp[:])
            st["c2T"] = c2T_new

        def lstep_branch(st):
            """Delayed h2 branch for the PREVIOUS step (never blocks chains)."""
            l = st["l"]
            if st["sall_prev"] is not None:
                tc2 = rp.tile([128, 128], BF16, tag=f"tc2{l}", name=f"tc2{l}")
                nc.scalar.activation(tc2[:], st["c2h_prev"][:], AF.Tanh)
                h2 = rp.tile([128, 128], BF16, tag=f"h2{l}", name=f"h2{l}")
                nc.gpsimd.tensor_mul(h2[:], st["sall_prev"][:, 256:384], tc2[:])
                tp2 = psp.tile([128, 128], BF16, tag=f"tp{l}", name=f"tp{l}",
                               bufs=2)
                nc.tensor.transpose(tp2[:], h2[:], identb[:])
                src_ = tp2[:].rearrange("p (k r) -> p k r", k=4)[:, :, 0:BS]
                dst = hT[l][:].rearrange(
                    "p (k b t) -> p k b t", k=4, b=BS)[:, :, :, st["tprev"]]
                nc.vector.tensor_copy(dst, src_)
            st["sall_prev"] = st["sall"]
            st["c2h_prev"] = st["c2h"]
            st["tprev"] = st["tprev"] + 1

        # phase-E operands: transferred during the recurrence pass
        # phase-E operands: transferred during the recurrence pass
        wint = pf.tile([128, 4 * DS], BF16, tag="wint", name="wint")
        nc.sync.dma_start(
            wint[:].rearrange("p (k c) -> p k c", k=4),
            WINT.ap().rearrange("(k p) c -> p k c", k=4))
        encb = pf.tile([S, 4 * DS], BF16, tag="encb", name="encb")
        nc.sync.dma_start(
            encb[:].rearrange("p (b c) -> p b c", b=BS),
            ENC.ap().rearrange("b s d -> s b d"))
        enctb = pf.tile([128, BS * 8 * S], BF16, tag="enctb", name="enctb")
        nc.sync.dma_start(
            enctb[:].rearrange("p (b k s) -> p b k s", b=BS, k=8),
            ENCT.ap().rearrange("b (k p) s -> p b k s", k=8))
        offsb = pf.tile([T, BS * S], F32, tag="offsb", name="offsb")
        nc.sync.dma_start(
            offsb[:].rearrange("p (b s) -> p b s", b=BS),
            OFFS.ap().rearrange("b t s -> t b s"))
        woutt = pf.tile([128, 12 * D], BF16, tag="woutt", name="woutt")
        nc.sync.dma_start(
            woutt[:].rearrange("p (k c) -> p k c", k=12),
            WOUTT.ap()[0:1536].rearrange("(k p) c -> p k c", k=12))
        woutb = pf.tile([1, D], BF16, tag="woutb", name="woutb")
        nc.sync.dma_start(woutb[:], WOUTT.ap()[1536:1537, :])

        st0 = linit(0)
        st1 = linit(1)
        for ss in range(T + LAG + 1):
            if ss < T:
                lstep_mm(st0, ss)                # PE: inj + rounds L0(t)
            if LAG < ss <= T + LAG:
                lstep_transpose(st1)             # PE: T_c2 L1(t'-1) (ready)
            if ss < T:
                lstep_sigma(st0, ss)
            if LAG <= ss < T + LAG:
                lstep_mm(st1, ss - LAG)          # PE: bias/xpart/whh L1(t')
            if LAG < ss <= T + LAG:
                lstep_branch(st1)                # PE: T_h2 L1(t'-1)
            if LAG <= ss < T + LAG:
                lstep_sigma(st1, ss - LAG)
            if ss < T:
                lstep_transpose(st0)             # PE: T_c2 L0(t) (ready)
            if ss <= T:
                lstep_branch(st0)                # PE: T_h2 L0(t-1)
        lstep_branch(st1)                        # flush T_h2 L1(63)

        # ---------------- Phase E: attention + out proj ----------------
        wkt2 = pa.tile([128, 4 * G], BF16, tag="wkt", name="wkt2")
        with tc.tile_pool(name="pe", bufs=1) as pe:
            sT = [hT[1][:, 256 * k:256 * (k + 1)] for k in range(4)]

            xqT = []
            for m in range(8):
                ps = gtile(m % 2, [128, R])
                for k in range(4):
                    nc.tensor.matmul(
                        ps[:], wint[:, 1024 * k + 128 * m:1024 * k + 128 * (m + 1)],
                        sT[k], start=(k == 0), stop=(k == 3))
                xq = wkt2[:, 256 * m:256 * (m + 1)]
                if m % 2 == 0:
                    nc.scalar.copy(xq, ps[:])
                else:
                    nc.vector.tensor_copy(xq, ps[:])
                xqT.append(xq)

            ctxT = [wkt2[:, 2048 + 256 * m:2048 + 256 * (m + 1)]
                    for m in range(8)]
            for b in range(BS):
                bsl = slice(T * b, T * (b + 1))
                eps = psp.tile([T, S], F32, tag=f"g{b % 2}",
                               name=f"g{b % 2}", bufs=2)
                for k in range(8):
                    nc.tensor.matmul(
                        eps[:], xqT[k][:, bsl],
                        enctb[:, 1024 * b + 128 * k:1024 * b + 128 * (k + 1)],
                        start=(k == 0), stop=(k == 7))
                esb = pe.tile([T, S], F32, tag=f"esb{b % 2}",
                              name=f"esb{b % 2}")
                nc.vector.tensor_add(esb[:], eps[:],
                                     offsb[:, 128 * b:128 * (b + 1)])
                negmax = pe.tile([T, 1], F32, tag=f"negmax{b % 2}",
                                 name=f"negmax{b % 2}")
                nc.vector.reduce_max(
                    negmax[:], esb[:], axis=mybir.AxisListType.X, negate=True)
                expE = pe.tile([T, S], F32, tag=f"expE{b % 2}",
                               name=f"expE{b % 2}")
                den = pe.tile([T, 1], F32, tag=f"den{b % 2}",
                              name=f"den{b % 2}")
                nc.scalar.activation(
                    expE[:], esb[:], AF.Exp, bias=negmax[:], accum_out=den[:])
                rden = pe.tile([T, 1], F32, tag=f"rden{b % 2}",
                               name=f"rden{b % 2}")
                nc.vector.reciprocal(rden[:], den[:])
                attn = pe.tile([T, S], F32, tag=f"attn{b % 2}",
                               name=f"attn{b % 2}")
                nc.vector.tensor_scalar_mul(attn[:], expE[:], rden[:])
                tpa = psp.tile([S, T], F32, tag=f"tp{b % 2}",
                               name=f"tp{b % 2}", bufs=2)
                nc.tensor.transpose(tpa[:], attn[:], ident[0:T, 0:T])
                atsb = pe.tile([S, T], BF16, tag=f"atsb{b % 2}",
                               name=f"atsb{b % 2}")
                nc.vector.tensor_copy(atsb[:], tpa[:])
                for m in range(8):
                    psc = gtile(m % 2, [128, T])
                    nc.tensor.matmul(
                        psc[:], encb[:, 1024 * b + 128 * m:1024 * b + 128 * (m + 1)],
                        atsb[:], start=True, stop=True)
                    if m % 2 == 0:
                        nc.scalar.copy(ctxT[m][:, bsl], psc[:])
                    else:
                        nc.vector.tensor_copy(ctxT[m][:, bsl], psc[:])

            outflat = OUT.ap().rearrange("b t d -> (b t) d")
            lhs_all = ctxT + sT + [ones]
            wt_all = [woutt[:, 512 * k:512 * (k + 1)] for k in range(12)] \
                + [woutb[:]]
            for mc in range(2):
                msl = slice(128 * mc, 128 * (mc + 1))
                ps = gtile(mc, [128, D])
                for k in range(13):
                    nc.tensor.matmul(
                        ps[:], lhs_all[k][:, msl], wt_all[k],
                        start=(k == 0), stop=(k == 12))
                osb = pa.tile([128, D], F32, tag=f"stg{mc}", name=f"osb{mc}")
                nc.scalar.activation(osb[:], ps[:], AF.Tanh)
                nc.sync.dma_start(outflat[msl, :], osb[:])

    nc.compile()
    return nc


def assemble(results):
    full = np.concatenate([r["out"] for r in results], axis=0)  # [B, T, D]
    outs = full.transpose(1, 0, 2)                              # [T, B, D]
    return np.ascontiguousarray(outs.reshape(-1, D).reshape(-1, T, D))


_nc_cache = None


def kernel(**inputs):
    global _nc_cache
    in_maps = host_prep(inputs)
    if _nc_cache is None:
        _nc_cache = build_program()
    res = run_bass_kernel_spmd(_nc_cache, in_maps, list(range(NCORES)))
    return assemble(res.results)


# revision 26
# speedup vs baseline: 1.0468x; 1.0430x over previous
"""Trainium2 Bass kernel for nn_Decoder_46042049413334.

Buggy 2-layer LSTM decoder with attention (B=32, T=64, S=128, D=512).

Structure (per core, batch sharded 8 ways, BS=4 examples/core):
  Phase A: xpart0 = [emb(tokens), 1] @ [W_ih0.T; b0]  -> XPsb0 (SBUF)
  Interleaved pass: layer-0 step t and layer-1 step t-2 run together;
    layer-1's xpart is accumulated per step directly into its gates
    PSUM from the transposed h2 history (hT0), so the PE stays busy
    enough to hold the HAM clock gate open (K=8/8).
  Phase E: attention + out-projection from hT1

Recurrence layout: gates PSUM [128, 512] where partition 32*j+b holds
(example b, d-block j) and the 512 free cols are {i,f,o,2g}x128 for
that d-block (g columns pre-scaled by 2 so tanh(g) = 2*sigmoid(2g)-1
comes out of a single full-width sigmoid). The four d-blocks' weight
streams run CONCURRENTLY in the PE array via tile_position=(0, 32*j)
column tiling. Elementwise runs once over all 128 partitions; c2 and
h2 are re-transposed per step ([128,128] PE transpose). Off-chain work
(tanh(c2), h2, its transpose/gather) is emitted one step late so no
engine FIFO ever blocks the recurrence chains.

Row ordering is b-major everywhere: row r = b_local*T + t.
"""
import numpy as np
import ml_dtypes
from contextlib import ExitStack

import concourse.bass as bass
import concourse.bacc as bacc
import concourse.tile as tile
from concourse import mybir, masks
from concourse.bass_utils import run_bass_kernel_spmd

F32 = mybir.dt.float32
BF16 = mybir.dt.bfloat16
AF = mybir.ActivationFunctionType
NPBF = ml_dtypes.bfloat16

B, T, S, D, L, V = 32, 64, 128, 512, 2, 32000
G = 4 * D        # 2048
DS = 2 * D       # 1024
NCORES = 8
BS = B // NCORES  # 4
R = BS * T        # 256 rows per core
LAG = 2          # layer-1 recurrence lag behind layer 0


# ---------------------------------------------------------------- host side

def _gate_perm():
    perm = np.zeros(G, dtype=np.int64)
    base = {0: 0, 1: 512, 2: 1536, 3: 1024}  # i, f, o, g
    for j in range(G):
        nb, pos = divmod(j, 512)
        sub, dd = divmod(pos, 128)
        perm[j] = base[sub] + nb * 128 + dd
    return perm


def host_prep(inputs):
    """Build the 8 per-core input maps (layout/gather work only)."""
    perm = _gate_perm()
    tokens = np.asarray(inputs["prev_tgt_tokens"])
    embed = np.asarray(inputs["embed"], dtype=np.float32)
    enc = np.asarray(inputs["encoder_out"], dtype=np.float32)
    mask = np.asarray(inputs["src_mask"])
    hid = np.asarray(inputs["hiddens"], dtype=np.float32)
    cells = np.asarray(inputs["cells"], dtype=np.float32)
    W_ih = np.asarray(inputs["W_ih"], dtype=np.float32)
    W_hh = np.asarray(inputs["W_hh"], dtype=np.float32)
    b_ih = np.asarray(inputs["b_ih"], dtype=np.float32)
    b_hh = np.asarray(inputs["b_hh"], dtype=np.float32)
    W_in = np.asarray(inputs["W_in"], dtype=np.float32)
    b_in = np.asarray(inputs["b_in"], dtype=np.float32)
    W_out = np.asarray(inputs["W_out"], dtype=np.float32)
    b_out = np.asarray(inputs["b_out"], dtype=np.float32)

    def bf(x):
        return np.ascontiguousarray(x, dtype=NPBF)

    # layer 0 keeps sub-order [i,f,o,2g]; layer 1 uses [i,f,2g,o] so the
    # in-loop stream is the contiguous 384 cols {i,f,2g} and o defers.
    perm1 = np.zeros(G, dtype=np.int64)
    base1 = {0: 0, 1: 512, 2: 1024, 3: 1536}  # i, f, g, o
    for jj in range(G):
        nb_, pos = divmod(jj, 512)
        sub, dd = divmod(pos, 128)
        perm1[jj] = base1[sub] + nb_ * 128 + dd
    WIH = []
    WHH = []
    gscale = np.ones(G, np.float32)
    gscale1 = np.ones(G, np.float32)
    for nb in range(4):
        gscale[512 * nb + 384:512 * (nb + 1)] = 2.0   # tanh(g)=2*sig(2g)-1
        gscale1[512 * nb + 256:512 * nb + 384] = 2.0
    for l, pm, gs in ((0, perm, gscale), (1, perm1, gscale1)):
        wihT = W_ih[l].T[:, pm] * gs
        biasrow = ((b_ih[l] + b_hh[l])[pm] * gs)[None, :]
        WIH.append(bf(np.concatenate([wihT, biasrow], 0)))   # [513, 2048]
        WHH.append(bf(W_hh[l].T[:, pm] * gs))                # [512, 2048]
    # layer-1 o-gate weights, block-major od = j*128+d
    osel = np.concatenate([np.arange(1536 + 128 * j, 1536 + 128 * (j + 1))
                           for j in range(4)])
    WIHO = bf(np.concatenate(
        [W_ih[1].T[:, osel],
         (b_ih[1] + b_hh[1])[osel][None, :]], 0))            # [513, 512]
    WHHO = bf(W_hh[1].T[:, osel])                            # [512, 512]
    WINT = bf(W_in.T)                                        # [512, 1024]
    WOUTT = bf(np.concatenate([W_out.T, b_out[None, :]], 0))  # [1537, 512]

    # xpart0 injection selectors, one per u = t%8:
    # XPsb0 partition (4j+b)*8+u feeds gates row 32j+b
    einj8 = np.zeros((8, 128, 128), np.float32)
    for u in range(8):
        for j in range(4):
            for b in range(BS):
                einj8[u, (4 * j + b) * 8 + u, 32 * j + b] = 1.0
    einj8 = bf(einj8.transpose(1, 0, 2).reshape(128, 8 * 128))

    # block selector: e4blk[j, p] = 1 iff p // 32 == j
    e4 = np.zeros((4, 128), np.float32)
    for j in range(4):
        e4[j, 32 * j:32 * (j + 1)] = 1.0
    e4 = bf(e4)

    in_maps = []
    for core in range(NCORES):
        bsl = slice(core * BS, (core + 1) * BS)
        xe = embed[tokens[bsl]]                              # [BS, T, D]
        Xaug = np.concatenate(
            [xe.reshape(R, D), np.ones((R, 1), np.float32)], axis=1)
        # permute rows so phase-A store DMAs land partition-parallel:
        # new row b*64 + (t%8)*8 + t//8  <- (b, t)
        rperm = np.zeros(R, np.int64)
        for b in range(BS):
            for t in range(T):
                rperm[b * T + (t % 8) * 8 + t // 8] = b * T + t
        XT0 = bf(Xaug[rperm].T)                              # [513, 256]
        enc_c = np.ascontiguousarray(enc[bsl])               # [BS, 128, 1024]
        encT_c = np.swapaxes(enc_c, 1, 2)                    # [BS, 1024, 128]
        offs = np.einsum("bsd,d->bs", enc_c, b_in) + np.where(mask[bsl], -1e9, 0.0)
        offs_rep = np.ascontiguousarray(
            np.broadcast_to(offs[:, None, :], (BS, T, S)), dtype=np.float32)
        # initial c2T: c2t0[l, p, 32k+b] = hid[l, b, 128k+p]
        th = hid[:, bsl].reshape(L, BS, 4, 128).transpose(0, 3, 2, 1)  # [L,128,4,BS]
        c2t0 = np.zeros((L, 128, 4, 32), np.float32)
        c2t0[:, :, :, 0:BS] = th
        c2t0 = bf(c2t0.reshape(L, 128, 128))
        # cells in partition layout: cellsp[l, 32j+b, p] = cells[l, b, 128j+p]
        tc_ = cells[:, bsl].reshape(L, BS, 4, 128).transpose(0, 2, 1, 3)  # [L,4,BS,128]
        cellsp = np.zeros((L, 4, 32, 128), np.float32)
        cellsp[:, :, 0:BS, :] = tc_
        cellsp = bf(cellsp.reshape(L, 128, 128))
        in_maps.append({
            "xt0": XT0,
            "wih0": WIH[0], "whh0": WHH[0],
            "wih1": WIH[1], "whh1": WHH[1],
            "wiho": WIHO, "whho": WHHO,
            "wint": WINT, "woutt": WOUTT,
            "enc": bf(enc_c), "enct": bf(encT_c), "offs": offs_rep,
            "c2t0": c2t0, "cellsp": cellsp,
            "ones1": np.ones((1, R), NPBF),
            "einj8": einj8, "e4blk": e4,
        })
    return in_maps


# ------------------------------------------------------------- device build

def build_program():
    nc = bacc.Bacc("TRN2", target_bir_lowering=False, debug=False)

    XT0 = nc.dram_tensor("xt0", [513, R], BF16, kind="ExternalInput")
    WIH0 = nc.dram_tensor("wih0", [513, G], BF16, kind="ExternalInput")
    WHH0 = nc.dram_tensor("whh0", [D, G], BF16, kind="ExternalInput")
    WIH1 = nc.dram_tensor("wih1", [513, G], BF16, kind="ExternalInput")
    WHH1 = nc.dram_tensor("whh1", [D, G], BF16, kind="ExternalInput")
    WIHO = nc.dram_tensor("wiho", [513, 512], BF16, kind="ExternalInput")
    WHHO = nc.dram_tensor("whho", [D, 512], BF16, kind="ExternalInput")
    WINT = nc.dram_tensor("wint", [D, DS], BF16, kind="ExternalInput")
    WOUTT = nc.dram_tensor("woutt", [DS + D + 1, D], BF16, kind="ExternalInput")
    ENC = nc.dram_tensor("enc", [BS, S, DS], BF16, kind="ExternalInput")
    ENCT = nc.dram_tensor("enct", [BS, DS, S], BF16, kind="ExternalInput")
    OFFS = nc.dram_tensor("offs", [BS, T, S], F32, kind="ExternalInput")
    C2T0 = nc.dram_tensor("c2t0", [L, 128, 128], BF16, kind="ExternalInput")
    CELLSP = nc.dram_tensor("cellsp", [L, 128, 128], BF16, kind="ExternalInput")
    ONES1 = nc.dram_tensor("ones1", [1, R], BF16, kind="ExternalInput")
    EINJ = nc.dram_tensor("einj8", [128, 8 * 128], BF16, kind="ExternalInput")
    E4BLK = nc.dram_tensor("e4blk", [4, 128], BF16, kind="ExternalInput")
    OUT = nc.dram_tensor("out", [BS, T, D], F32, kind="ExternalOutput")

    with tile.TileContext(nc) as tc, ExitStack() as ctx:
        cpool = ctx.enter_context(tc.tile_pool(name="const", bufs=1))
        ident = cpool.tile([128, 128], F32)
        masks.make_identity(nc, ident[:])
        identb = cpool.tile([128, 128], BF16, name="identb")
        masks.make_identity(nc, identb[:])
        ones = cpool.tile([1, R], BF16, name="ones")
        nc.sync.dma_start(ones[:], ONES1.ap())
        einj8 = cpool.tile([128, 8 * 128], BF16, name="einj8")
        nc.sync.dma_start(einj8[:], EINJ.ap())
        e4blk = cpool.tile([4, 128], BF16, name="e4blk")
        nc.sync.dma_start(e4blk[:], E4BLK.ap())
        ones128 = cpool.tile([128, 128], BF16, name="ones128")
        nc.gpsimd.memset(ones128[:], 1.0)
        tw = cpool.tile([1, 4], F32, name="tw")
        nc.scalar.activation(tw[:], ones128[0:1, 0:4], AF.Sigmoid)
        nc.scalar.activation(tw[:], ones128[0:1, 0:4], AF.Tanh)
        nc.scalar.activation(tw[:], ones128[0:1, 0:4], AF.Exp)

        psp = ctx.enter_context(tc.tile_pool(name="ps", bufs=1, space="PSUM"))

        def gtile(idx, shape):
            return psp.tile(shape, F32, tag=f"g{idx}", name=f"g{idx}",
                            bufs=2 if idx < 2 else 1)

        # persistent SBUF xpart0:
        # XPsb0[(4*nb+b)*8 + t%8, (t//8)*512 + c] = xpart0[b,t,512nb+c]
        xpp = ctx.enter_context(tc.tile_pool(name="xps", bufs=1))
        XPsb0 = xpp.tile([128, (T // 8) * 512], BF16, name="xpsb0")

        # transposed histories: hT0 = h2_0^T; hT1 = c2_1^T (+1 shifted);
        # tanhH = tanh(c2_1)^T; sTT = h2_1^T (built post-pass)
        hT = [xpp.tile([128, 4 * R], BF16, name=f"hT{l}") for l in range(L)]
        tanhH = xpp.tile([128, 4 * R], BF16, name="tanhH")
        sTT = xpp.tile([128, 4 * R], BF16, name="sTT")

        # ---------------- Phase A inputs (packed, few DMAs) ----------------
        pa = ctx.enter_context(tc.tile_pool(name="pa", bufs=1))
        xtt = pa.tile([128, 4 * R], BF16, tag="xtt", name="xtt")
        nc.sync.dma_start(
            xtt[:].rearrange("p (k c) -> p k c", k=4),
            XT0.ap()[0:512].rearrange("(k p) c -> p k c", k=4))
        xt4 = pa.tile([1, R], BF16, tag="xt4", name="xt4")
        nc.sync.dma_start(xt4[:], XT0.ap()[512:513, :])

        # PE warm-up: dummy matmuls on the identity while DMAs land
        wps = psp.tile([128, 128], F32, tag="g0", name="g0", bufs=2)
        for w in range(48):
            nc.tensor.matmul(wps[:], identb[:], identb[:],
                             start=True, stop=True, skip_group_check=True)

        # prefetch pool: recurrence weights + attention operands
        pf = ctx.enter_context(tc.tile_pool(name="pf", bufs=1))
        cTb = pf.tile([128, 2 * 128], BF16, tag="cTb", name="cTb")
        nc.sync.dma_start(
            cTb[:].rearrange("p (l c) -> p l c", l=2),
            CELLSP.ap().rearrange("l p c -> p l c"))
        c2T0b = pf.tile([128, 2 * 128], BF16, tag="c2T0b", name="c2T0b")
        nc.sync.dma_start(
            c2T0b[:].rearrange("p (l c) -> p l c", l=2),
            C2T0.ap().rearrange("l p c -> p l c"))
        wkt = pa.tile([128, 4 * G], BF16, tag="wkt", name="wkt")
        for k in range(4):
            nc.sync.dma_start(
                wkt[:, 2048 * k:2048 * (k + 1)],
                WIH0.ap()[128 * k:128 * (k + 1), :])
        wk4 = pa.tile([1, G], BF16, tag="wk4", name="wk4")
        nc.sync.dma_start(wk4[:], WIH0.ap()[512:513, :])
        whht = []
        for l, Wd in ((0, WHH0), (1, WHH1)):
            wt = pf.tile([128, 4 * G], BF16, tag=f"whh{l}", name=f"whh{l}")
            for k in range(4):
                nc.sync.dma_start(
                    wt[:, 2048 * k:2048 * (k + 1)],
                    Wd.ap()[128 * k:128 * (k + 1), :])
            whht.append(wt)
        wihC = pf.tile([128, 4 * G], BF16, tag="wihC", name="wihC")
        nc.sync.dma_start(
            wihC[:].rearrange("p (k c) -> p k c", k=4),
            WIH1.ap()[0:512].rearrange("(k p) c -> p k c", k=4))
        wihC4 = pf.tile([4, 512], BF16, tag="wihC4", name="wihC4")
        nc.sync.dma_start(wihC4[:], WIH1.ap()[512:513, :].rearrange(
            "a (j c) -> (a j) c", j=4))
        # ---------------- Phase A: xpart0 ----------------
        for mc in range(2):
            for nb in range(4):
                ps = gtile(nb % 2, [128, 512])
                for k in range(4):
                    nc.tensor.matmul(
                        ps[:],
                        xtt[:, 256 * k + 128 * mc:256 * k + 128 * (mc + 1)],
                        wkt[:, 2048 * k + 512 * nb:2048 * k + 512 * (nb + 1)],
                        start=(k == 0), stop=False)
                nc.tensor.matmul(
                    ps[:], xt4[:, 128 * mc:128 * (mc + 1)],
                    wk4[:, 512 * nb:512 * (nb + 1)],
                    start=False, stop=True)
                sb = pa.tile([128, 512], BF16, tag=f"stg{nb}", name=f"stg{nb}")
                nc.vector.tensor_copy(sb[:], ps[:])
                p0 = (4 * nb + 2 * mc) * 8
                dst = XPsb0[p0:p0 + 16, :].rearrange(
                    "p (q c) -> p q c", c=512)
                nc.sync.dma_start(dst, sb[:])

        # ---------------- Interleaved recurrence passes ----------------
        rp = ctx.enter_context(tc.tile_pool(name="rp", bufs=2))

        def linit(l):
            return {"l": l, "cT": cTb[:, 128 * l:128 * (l + 1)],
                    "c2T": c2T0b[:, 128 * l:128 * (l + 1)], "whh": whht[l],
                    "sall_prev": None, "c2h_prev": None, "tprev": -1}

        def lstep_mm(st, t):
            """Inject/xpart + W_hh rounds for step t (PE bulk)."""
            l = st["l"]
            gates = gtile(l, [128, 512 if l == 0 else 384])
            if l == 0:
                rhs = XPsb0[:, 512 * (t // 8):512 * (t // 8 + 1)]
                nc.tensor.matmul(
                    gates[:], einj8[:, 128 * (t % 8):128 * (t % 8 + 1)], rhs,
                    start=True, stop=False, skip_group_check=True)
            else:
                # xpart1(t) accumulated in place: bias row first (writes all
                # 128 partitions), then W_ih1 rounds from hT0 columns of t
                nc.tensor.matmul(gates[:], e4blk[:], wihC4[:, 0:384],
                                 start=True, stop=False, skip_group_check=True)
                for k in range(4):
                    lhsT = hT[0][:].rearrange(
                        "p (k b t) -> p k b t", k=4, b=BS)[:, k, :, t]
                    for j in range(4):
                        nc.tensor.matmul(
                            gates[32 * j:32 * j + BS, :],
                            lhsT,
                            wihC[:, 2048 * k + 512 * j:2048 * k + 512 * j + 384],
                            start=False, stop=False,
                            tile_position=(0, 32 * j), skip_group_check=True)
            w = 512 if l == 0 else 384
            for k in range(4):
                lhsT = st["c2T"][:, 32 * k:32 * k + BS]
                for j in range(4):
                    nc.tensor.matmul(
                        gates[32 * j:32 * j + BS, :],
                        lhsT,
                        st["whh"][:, 2048 * k + 512 * j:2048 * k + 512 * j + w],
                        start=False, stop=(k == 3),
                        tile_position=(0, 32 * j), skip_group_check=True)
            st["gates"] = gates

        def lstep_sigma(st, t):
            """sigma + c2 elementwise chain for step t (no transpose)."""
            l = st["l"]
            gates = st["gates"]
            w = 512 if l == 0 else 384
            sall = rp.tile([128, w], F32, tag=f"sa{l}", name=f"sa{l}",
                           bufs=3)
            nc.scalar.activation(sall[:], gates[:], AF.Sigmoid)
            m1 = rp.tile([128, 128], BF16, tag=f"m1{l}", name=f"m1{l}")
            nc.gpsimd.tensor_mul(m1[:], sall[:, 128:256], st["cT"])
            tgv = rp.tile([128, 128], BF16, tag=f"tg{l}", name=f"tg{l}")
            nc.vector.scalar_tensor_tensor(
                tgv[:], sall[:, w - 128:w], 2.0, ones128[:],
                mybir.AluOpType.mult, mybir.AluOpType.subtract)
            m2 = rp.tile([128, 128], BF16, tag=f"m2{l}", name=f"m2{l}")
            nc.vector.tensor_mul(m2[:], sall[:, 0:128], tgv[:])
            c2h = rp.tile([128, 128], BF16, tag=f"c2h{l}", name=f"c2h{l}",
                          bufs=3)
            nc.vector.tensor_add(c2h[:], m1[:], m2[:])
            st["sall"] = sall
            st["c2h"] = c2h

        def lstep_transpose(st, t):
            """c2 transpose + cast — emitted at a PE-queue position where
            the chain dependency (add) has already resolved."""
            l = st["l"]
            tp = psp.tile([128, 128], BF16, tag=f"tp{l}", name=f"tp{l}",
                          bufs=2)
            nc.tensor.transpose(tp[:], st["c2h"][:], identb[:])
            c2T_new = rp.tile([128, 128], BF16, tag=f"c2T{l}", name=f"c2T{l}")
            nc.vector.tensor_copy(c2T_new[:], tp[:])
            st["c2T"] = c2T_new
            if l == 1:
                # histories for the deferred o-gate / h2_1 reconstruction
                tview = tp[:].rearrange("p (k r) -> p k r", k=4)[:, :, 0:BS]
                if t < T - 1:
                    dstc = hT[1][:].rearrange(
                        "p (k b t) -> p k b t", k=4, b=BS)[:, :, :, t + 1]
                    nc.vector.tensor_copy(dstc, tview)
                th = rp.tile([128, 128], BF16, tag="th1", name="th1")
                nc.scalar.activation(th[:], tp[:], AF.Tanh)
                dstt = tanhH[:].rearrange(
                    "p (k b t) -> p k b t", k=4, b=BS)[:, :, :, t]
                nc.vector.tensor_copy(
                    dstt, th[:].rearrange("p (k r) -> p k r", k=4)[:, :, 0:BS])

        def lstep_branch(st):
            """Delayed h2 branch for the PREVIOUS step (never blocks chains)."""
            l = st["l"]
            if l == 1:
                return
            if st["sall_prev"] is not None:
                tc2 = rp.tile([128, 128], BF16, tag=f"tc2{l}", name=f"tc2{l}")
                nc.scalar.activation(tc2[:], st["c2h_prev"][:], AF.Tanh)
                h2 = rp.tile([128, 128], BF16, tag=f"h2{l}", name=f"h2{l}")
                nc.gpsimd.tensor_mul(h2[:], st["sall_prev"][:, 256:384], tc2[:])
                tp2 = psp.tile([128, 128], BF16, tag=f"tp{l}", name=f"tp{l}",
                               bufs=2)
                nc.tensor.transpose(tp2[:], h2[:], identb[:])
                src_ = tp2[:].rearrange("p (k r) -> p k r", k=4)[:, :, 0:BS]
                dst = hT[l][:].rearrange(
                    "p (k b t) -> p k b t", k=4, b=BS)[:, :, :, st["tprev"]]
                nc.vector.tensor_copy(dst, src_)
            st["sall_prev"] = st["sall"]
            st["c2h_prev"] = st["c2h"]
            st["tprev"] = st["tprev"] + 1

        # phase-E operands: transferred during the recurrence pass
        # phase-E operands: transferred during the recurrence pass
        wiho = pf.tile([128, 4 * 512], BF16, tag="wiho", name="wiho")
        nc.sync.dma_start(
            wiho[:].rearrange("p (k c) -> p k c", k=4),
            WIHO.ap()[0:512].rearrange("(k p) c -> p k c", k=4))
        wihob = pf.tile([1, 512], BF16, tag="wihob", name="wihob")
        nc.sync.dma_start(wihob[:], WIHO.ap()[512:513, :])
        whho = pf.tile([128, 4 * 512], BF16, tag="whho", name="whho")
        nc.sync.dma_start(
            whho[:].rearrange("p (k c) -> p k c", k=4),
            WHHO.ap().rearrange("(k p) c -> p k c", k=4))
        wint = pf.tile([128, 4 * DS], BF16, tag="wint", name="wint")
        nc.sync.dma_start(
            wint[:].rearrange("p (k c) -> p k c", k=4),
            WINT.ap().rearrange("(k p) c -> p k c", k=4))
        encb = pf.tile([S, 4 * DS], BF16, tag="encb", name="encb")
        nc.sync.dma_start(
            encb[:].rearrange("p (b c) -> p b c", b=BS),
            ENC.ap().rearrange("b s d -> s b d"))
        enctb = pf.tile([128, BS * 8 * S], BF16, tag="enctb", name="enctb")
        nc.sync.dma_start(
            enctb[:].rearrange("p (b k s) -> p b k s", b=BS, k=8),
            ENCT.ap().rearrange("b (k p) s -> p b k s", k=8))
        offsb = pf.tile([T, BS * S], F32, tag="offsb", name="offsb")
        nc.sync.dma_start(
            offsb[:].rearrange("p (b s) -> p b s", b=BS),
            OFFS.ap().rearrange("b t s -> t b s"))
        woutt = pf.tile([128, 12 * D], BF16, tag="woutt", name="woutt")
        nc.sync.dma_start(
            woutt[:].rearrange("p (k c) -> p k c", k=12),
            WOUTT.ap()[0:1536].rearrange("(k p) c -> p k c", k=12))
        woutb = pf.tile([1, D], BF16, tag="woutb", name="woutb")
        nc.sync.dma_start(woutb[:], WOUTT.ap()[1536:1537, :])

        st0 = linit(0)
        st1 = linit(1)
        # hT1 col 0 = initial c2_1
        nc.vector.tensor_copy(
            hT[1][:].rearrange("p (k b t) -> p k b t", k=4, b=BS)[:, :, :, 0],
            c2T0b[:, 128:256].rearrange("p (k r) -> p k r", k=4)[:, :, 0:BS])
        for ss in range(T + LAG + 1):
            if ss < T:
                lstep_mm(st0, ss)                # PE: inj + rounds L0(t)
            if LAG < ss <= T + LAG:
                lstep_transpose(st1, ss - LAG - 1)  # PE: T_c2 L1(t'-1)
            if ss < T:
                lstep_sigma(st0, ss)
            if LAG <= ss < T + LAG:
                lstep_mm(st1, ss - LAG)          # PE: bias/xpart/whh L1(t')
            if LAG <= ss < T + LAG:
                lstep_sigma(st1, ss - LAG)
            if ss < T:
                lstep_transpose(st0, ss)         # PE: T_c2 L0(t) (ready)
            if ss <= T:
                lstep_branch(st0)                # PE: T_h2 L0(t-1)

        # ---------------- deferred o-gate / h2_1 reconstruction ----------
        with tc.tile_pool(name="po", bufs=1) as po:
            for mc in range(2):
                msl = slice(128 * mc, 128 * (mc + 1))
                ps = gtile(mc, [128, 512])
                for k in range(4):
                    nc.tensor.matmul(
                        ps[:], hT[0][:, 256 * k + 128 * mc:
                                     256 * k + 128 * (mc + 1)],
                        wiho[:, 512 * k:512 * (k + 1)],
                        start=(k == 0), stop=False)
                for k in range(4):
                    nc.tensor.matmul(
                        ps[:], hT[1][:, 256 * k + 128 * mc:
                                     256 * k + 128 * (mc + 1)],
                        whho[:, 512 * k:512 * (k + 1)],
                        start=False, stop=False)
                nc.tensor.matmul(
                    ps[:], ones[0:1, 0:128], wihob[:],
                    start=False, stop=True)
                sob = po.tile([128, 512], BF16, tag=f"sob{mc}",
                              name=f"sob{mc}")
                nc.scalar.activation(sob[:], ps[:], AF.Sigmoid)
                for j in range(4):
                    tpo = psp.tile([128, 128], BF16, tag=f"tp{j % 2}",
                                   name=f"tp{j % 2}", bufs=2)
                    nc.tensor.transpose(
                        tpo[:], sob[:, 128 * j:128 * (j + 1)], identb[:])
                    dsl = slice(256 * j + 128 * mc, 256 * j + 128 * (mc + 1))
                    nc.vector.tensor_mul(
                        sTT[:, dsl], tpo[:], tanhH[:, dsl])

        # ---------------- Phase E: attention + out proj ----------------
        wkt2 = pa.tile([128, 4 * G], BF16, tag="wkt", name="wkt2")
        with tc.tile_pool(name="pe", bufs=1) as pe:
            sT = [sTT[:, 256 * k:256 * (k + 1)] for k in range(4)]

            xqT = []
            for m in range(8):
                ps = gtile(m % 2, [128, R])
                for k in range(4):
                    nc.tensor.matmul(
                        ps[:], wint[:, 1024 * k + 128 * m:1024 * k + 128 * (m + 1)],
                        sT[k], start=(k == 0), stop=(k == 3))
                xq = wkt2[:, 256 * m:256 * (m + 1)]
                if m % 2 == 0:
                    nc.scalar.copy(xq, ps[:])
                else:
                    nc.vector.tensor_copy(xq, ps[:])
                xqT.append(xq)

            ctxT = [wkt2[:, 2048 + 256 * m:2048 + 256 * (m + 1)]
                    for m in range(8)]
            for b in range(BS):
                bsl = slice(T * b, T * (b + 1))
                eps = psp.tile([T, S], F32, tag=f"g{b % 2}",
                               name=f"g{b % 2}", bufs=2)
                for k in range(8):
                    nc.tensor.matmul(
                        eps[:], xqT[k][:, bsl],
                        enctb[:, 1024 * b + 128 * k:1024 * b + 128 * (k + 1)],
                        start=(k == 0), stop=(k == 7))
                esb = pe.tile([T, S], F32, tag=f"esb{b % 2}",
                              name=f"esb{b % 2}")
                nc.vector.tensor_add(esb[:], eps[:],
                                     offsb[:, 128 * b:128 * (b + 1)])
                negmax = pe.tile([T, 1], F32, tag=f"negmax{b % 2}",
                                 name=f"negmax{b % 2}")
                nc.vector.reduce_max(
                    negmax[:], esb[:], axis=mybir.AxisListType.X, negate=True)
                expE = pe.tile([T, S], F32, tag=f"expE{b % 2}",
                               name=f"expE{b % 2}")
                den = pe.tile([T, 1], F32, tag=f"den{b % 2}",
                              name=f"den{b % 2}")
                nc.scalar.activation(
                    expE[:], esb[:], AF.Exp, bias=negmax[:], accum_out=den[:])
                rden = pe.tile([T, 1], F32, tag=f"rden{b % 2}",
                               name=f"rden{b % 2}")
                nc.vector.reciprocal(rden[:], den[:])
                attn = pe.tile([T, S], F32, tag=f"attn{b % 2}",
                               name=f"attn{b % 2}")
                nc.vector.tensor_scalar_mul(attn[:], expE[:], rden[:])
                tpa = psp.tile([S, T], F32, tag=f"tp{b % 2}",
                               name=f"tp{b % 2}", bufs=2)
                nc.tensor.transpose(tpa[:], attn[:], ident[0:T, 0:T])
                atsb = pe.tile([S, T], BF16, tag=f"atsb{b % 2}",
                               name=f"atsb{b % 2}")
                nc.vector.tensor_copy(atsb[:], tpa[:])
                for m in range(8):
                    psc = gtile(m % 2, [128, T])
                    nc.tensor.matmul(
                        psc[:], encb[:, 1024 * b + 128 * m:1024 * b + 128 * (m + 1)],
                        atsb[:], start=True, stop=True)
                    if m % 2 == 0:
                        nc.scalar.copy(ctxT[m][:, bsl], psc[:])
                    else:
                        nc.vector.tensor_copy(ctxT[m][:, bsl], psc[:])

            outflat = OUT.ap().rearrange("b t d -> (b t) d")
            lhs_all = ctxT + sT + [ones]
            wt_all = [woutt[:, 512 * k:512 * (k + 1)] for k in range(12)] \
                + [woutb[:]]
            for mc in range(2):
                msl = slice(128 * mc, 128 * (mc + 1))
                ps = gtile(mc, [128, D])
                for k in range(13):
                    nc.tensor.matmul(
                        ps[:], lhs_all[k][:, msl], wt_all[k],
                        start=(k == 0), stop=(k == 12))
                osb = pa.tile([128, D], F32, tag=f"stg{mc}", name=f"osb{mc}")
                nc.scalar.activation(osb[:], ps[:], AF.Tanh)
                nc.sync.dma_start(outflat[msl, :], osb[:])

    nc.compile()
    return nc


def assemble(results):
    full = np.concatenate([r["out"] for r in results], axis=0)  # [B, T, D]
    outs = full.transpose(1, 0, 2)                              # [T, B, D]
    return np.ascontiguousarray(outs.reshape(-1, D).reshape(-1, T, D))


_nc_cache = None


def kernel(**inputs):
    global _nc_cache
    in_maps = host_prep(inputs)
    if _nc_cache is None:
        _nc_cache = build_program()
    res = run_bass_kernel_spmd(_nc_cache, in_maps, list(range(NCORES)))
    return assemble(res.results)


# revision 27
# speedup vs baseline: 1.1096x; 1.0600x over previous
"""Trainium2 Bass kernel for nn_Decoder_46042049413334.

Buggy 2-layer LSTM decoder with attention (B=32, T=64, S=128, D=512).

Structure (per core, batch sharded 8 ways, BS=4 examples/core):
  Phase A: xpart0 = [emb(tokens), 1] @ [W_ih0.T; b0]  -> XPsb0 (SBUF)
  Interleaved pass: layer-0 step t and layer-1 step t-2 run together;
    layer-1's xpart is accumulated per step directly into its gates
    PSUM from the transposed h2 history (hT0), so the PE stays busy
    enough to hold the HAM clock gate open (K=8/8).
  Phase E: attention + out-projection from hT1

Recurrence layout: gates PSUM [128, 512] where partition 32*j+b holds
(example b, d-block j) and the 512 free cols are {i,f,o,2g}x128 for
that d-block (g columns pre-scaled by 2 so tanh(g) = 2*sigmoid(2g)-1
comes out of a single full-width sigmoid). The four d-blocks' weight
streams run CONCURRENTLY in the PE array via tile_position=(0, 32*j)
column tiling. Elementwise runs once over all 128 partitions; c2 and
h2 are re-transposed per step ([128,128] PE transpose). Off-chain work
(tanh(c2), h2, its transpose/gather) is emitted one step late so no
engine FIFO ever blocks the recurrence chains.

Row ordering is b-major everywhere: row r = b_local*T + t.
"""
import numpy as np
import ml_dtypes
from contextlib import ExitStack

import concourse.bass as bass
import concourse.bacc as bacc
import concourse.tile as tile
from concourse import mybir, masks
from concourse.bass_utils import run_bass_kernel_spmd

F32 = mybir.dt.float32
BF16 = mybir.dt.bfloat16
AF = mybir.ActivationFunctionType
NPBF = ml_dtypes.bfloat16

B, T, S, D, L, V = 32, 64, 128, 512, 2, 32000
G = 4 * D        # 2048
DS = 2 * D       # 1024
NCORES = 8
BS = B // NCORES  # 4
R = BS * T        # 256 rows per core
LAG = 2          # layer-1 recurrence lag behind layer 0


# ---------------------------------------------------------------- host side

def _gate_perm():
    perm = np.zeros(G, dtype=np.int64)
    base = {0: 0, 1: 512, 2: 1536, 3: 1024}  # i, f, o, g
    for j in range(G):
        nb, pos = divmod(j, 512)
        sub, dd = divmod(pos, 128)
        perm[j] = base[sub] + nb * 128 + dd
    return perm


def host_prep(inputs):
    """Build the 8 per-core input maps (layout/gather work only)."""
    perm = _gate_perm()
    tokens = np.asarray(inputs["prev_tgt_tokens"])
    embed = np.asarray(inputs["embed"], dtype=np.float32)
    enc = np.asarray(inputs["encoder_out"], dtype=np.float32)
    mask = np.asarray(inputs["src_mask"])
    hid = np.asarray(inputs["hiddens"], dtype=np.float32)
    cells = np.asarray(inputs["cells"], dtype=np.float32)
    W_ih = np.asarray(inputs["W_ih"], dtype=np.float32)
    W_hh = np.asarray(inputs["W_hh"], dtype=np.float32)
    b_ih = np.asarray(inputs["b_ih"], dtype=np.float32)
    b_hh = np.asarray(inputs["b_hh"], dtype=np.float32)
    W_in = np.asarray(inputs["W_in"], dtype=np.float32)
    b_in = np.asarray(inputs["b_in"], dtype=np.float32)
    W_out = np.asarray(inputs["W_out"], dtype=np.float32)
    b_out = np.asarray(inputs["b_out"], dtype=np.float32)

    def bf(x):
        return np.ascontiguousarray(x, dtype=NPBF)

    # layer 0 keeps sub-order [i,f,o,2g]; layer 1 uses [i,f,2g,o] so the
    # in-loop stream is the contiguous 384 cols {i,f,2g} and o defers.
    perm1 = np.zeros(G, dtype=np.int64)
    base1 = {0: 0, 1: 512, 2: 1024, 3: 1536}  # i, f, g, o
    for jj in range(G):
        nb_, pos = divmod(jj, 512)
        sub, dd = divmod(pos, 128)
        perm1[jj] = base1[sub] + nb_ * 128 + dd
    WIH = []
    WHH = []
    gscale = np.ones(G, np.float32)
    gscale1 = np.ones(G, np.float32)
    for nb in range(4):
        gscale[512 * nb + 384:512 * (nb + 1)] = 2.0   # tanh(g)=2*sig(2g)-1
        gscale1[512 * nb + 256:512 * nb + 384] = 2.0
    for l, pm, gs in ((0, perm1, gscale1), (1, perm1, gscale1)):
        wihT = W_ih[l].T[:, pm] * gs
        biasrow = ((b_ih[l] + b_hh[l])[pm] * gs)[None, :]
        WIH.append(bf(np.concatenate([wihT, biasrow], 0)))   # [513, 2048]
        WHH.append(bf(W_hh[l].T[:, pm] * gs))                # [512, 2048]
    # layer-1 o-gate weights, block-major od = j*128+d
    osel = np.concatenate([np.arange(1536 + 128 * j, 1536 + 128 * (j + 1))
                           for j in range(4)])
    WIHO = bf(np.concatenate(
        [W_ih[1].T[:, osel],
         (b_ih[1] + b_hh[1])[osel][None, :]], 0))            # [513, 512]
    WHHO = bf(W_hh[1].T[:, osel])                            # [512, 512]
    WINT = bf(W_in.T)                                        # [512, 1024]
    WOUTT = bf(np.concatenate([W_out.T, b_out[None, :]], 0))  # [1537, 512]

    # xpart0 injection selectors, one per u = t%8:
    # XPsb0 partition (4j+b)*8+u feeds gates row 32j+b
    einj8 = np.zeros((8, 128, 128), np.float32)
    for u in range(8):
        for j in range(4):
            for b in range(BS):
                einj8[u, (4 * j + b) * 8 + u, 32 * j + b] = 1.0
    einj8 = bf(einj8.transpose(1, 0, 2).reshape(128, 8 * 128))

    # block selector: e4blk[j, p] = 1 iff p // 32 == j
    e4 = np.zeros((4, 128), np.float32)
    for j in range(4):
        e4[j, 32 * j:32 * (j + 1)] = 1.0
    e4 = bf(e4)

    in_maps = []
    for core in range(NCORES):
        bsl = slice(core * BS, (core + 1) * BS)
        xe = embed[tokens[bsl]]                              # [BS, T, D]
        Xaug = np.concatenate(
            [xe.reshape(R, D), np.ones((R, 1), np.float32)], axis=1)
        # permute rows so phase-A store DMAs land partition-parallel:
        # new row b*64 + (t%8)*8 + t//8  <- (b, t)
        rperm = np.zeros(R, np.int64)
        for b in range(BS):
            for t in range(T):
                rperm[b * T + (t % 8) * 8 + t // 8] = b * T + t
        XT0 = bf(Xaug[rperm].T)                              # [513, 256]
        enc_c = np.ascontiguousarray(enc[bsl])               # [BS, 128, 1024]
        encT_c = np.swapaxes(enc_c, 1, 2)                    # [BS, 1024, 128]
        offs = np.einsum("bsd,d->bs", enc_c, b_in) + np.where(mask[bsl], -1e9, 0.0)
        offs_rep = np.ascontiguousarray(
            np.broadcast_to(offs[:, None, :], (BS, T, S)), dtype=np.float32)
        # initial c2T: c2t0[l, p, 32k+b] = hid[l, b, 128k+p]
        th = hid[:, bsl].reshape(L, BS, 4, 128).transpose(0, 3, 2, 1)  # [L,128,4,BS]
        c2t0 = np.zeros((L, 128, 4, 32), np.float32)
        c2t0[:, :, :, 0:BS] = th
        c2t0 = bf(c2t0.reshape(L, 128, 128))
        # cells in partition layout: cellsp[l, 32j+b, p] = cells[l, b, 128j+p]
        tc_ = cells[:, bsl].reshape(L, BS, 4, 128).transpose(0, 2, 1, 3)  # [L,4,BS,128]
        cellsp = np.zeros((L, 4, 32, 128), np.float32)
        cellsp[:, :, 0:BS, :] = tc_
        cellsp = bf(cellsp.reshape(L, 128, 128))
        in_maps.append({
            "xt0": XT0,
            "wih0": WIH[0], "whh0": WHH[0],
            "wih1": WIH[1], "whh1": WHH[1],
            "wiho": WIHO, "whho": WHHO,
            "wint": WINT, "woutt": WOUTT,
            "enc": bf(enc_c), "enct": bf(encT_c), "offs": offs_rep,
            "c2t0": c2t0, "cellsp": cellsp,
            "ones1": np.ones((1, R), NPBF),
            "einj8": einj8, "e4blk": e4,
        })
    return in_maps


# ------------------------------------------------------------- device build

def build_program():
    nc = bacc.Bacc("TRN2", target_bir_lowering=False, debug=False)

    XT0 = nc.dram_tensor("xt0", [513, R], BF16, kind="ExternalInput")
    WIH0 = nc.dram_tensor("wih0", [513, G], BF16, kind="ExternalInput")
    WHH0 = nc.dram_tensor("whh0", [D, G], BF16, kind="ExternalInput")
    WIH1 = nc.dram_tensor("wih1", [513, G], BF16, kind="ExternalInput")
    WHH1 = nc.dram_tensor("whh1", [D, G], BF16, kind="ExternalInput")
    WIHO = nc.dram_tensor("wiho", [513, 512], BF16, kind="ExternalInput")
    WHHO = nc.dram_tensor("whho", [D, 512], BF16, kind="ExternalInput")
    WINT = nc.dram_tensor("wint", [D, DS], BF16, kind="ExternalInput")
    WOUTT = nc.dram_tensor("woutt", [DS + D + 1, D], BF16, kind="ExternalInput")
    ENC = nc.dram_tensor("enc", [BS, S, DS], BF16, kind="ExternalInput")
    ENCT = nc.dram_tensor("enct", [BS, DS, S], BF16, kind="ExternalInput")
    OFFS = nc.dram_tensor("offs", [BS, T, S], F32, kind="ExternalInput")
    C2T0 = nc.dram_tensor("c2t0", [L, 128, 128], BF16, kind="ExternalInput")
    CELLSP = nc.dram_tensor("cellsp", [L, 128, 128], BF16, kind="ExternalInput")
    ONES1 = nc.dram_tensor("ones1", [1, R], BF16, kind="ExternalInput")
    EINJ = nc.dram_tensor("einj8", [128, 8 * 128], BF16, kind="ExternalInput")
    E4BLK = nc.dram_tensor("e4blk", [4, 128], BF16, kind="ExternalInput")
    OUT = nc.dram_tensor("out", [BS, T, D], F32, kind="ExternalOutput")

    with tile.TileContext(nc) as tc, ExitStack() as ctx:
        cpool = ctx.enter_context(tc.tile_pool(name="const", bufs=1))
        ident = cpool.tile([128, 128], F32)
        masks.make_identity(nc, ident[:])
        identb = cpool.tile([128, 128], BF16, name="identb")
        masks.make_identity(nc, identb[:])
        ones = cpool.tile([1, R], BF16, name="ones")
        nc.sync.dma_start(ones[:], ONES1.ap())
        einj8 = cpool.tile([128, 8 * 128], BF16, name="einj8")
        nc.sync.dma_start(einj8[:], EINJ.ap())
        e4blk = cpool.tile([4, 128], BF16, name="e4blk")
        nc.sync.dma_start(e4blk[:], E4BLK.ap())
        ones128 = cpool.tile([128, 128], BF16, name="ones128")
        nc.gpsimd.memset(ones128[:], 1.0)
        tw = cpool.tile([1, 4], F32, name="tw")
        nc.scalar.activation(tw[:], ones128[0:1, 0:4], AF.Sigmoid)
        nc.scalar.activation(tw[:], ones128[0:1, 0:4], AF.Tanh)
        nc.scalar.activation(tw[:], ones128[0:1, 0:4], AF.Exp)

        psp = ctx.enter_context(tc.tile_pool(name="ps", bufs=1, space="PSUM"))

        def gtile(idx, shape):
            return psp.tile(shape, F32, tag=f"g{idx}", name=f"g{idx}",
                            bufs=2 if idx < 2 else 1)

        # persistent SBUF xpart0:
        # XPsb0[(4*nb+b)*8 + t%8, (t//8)*512 + c] = xpart0[b,t,512nb+c]
        xpp = ctx.enter_context(tc.tile_pool(name="xps", bufs=1))
        XPsb0 = xpp.tile([128, (T // 8) * 512], BF16, name="xpsb0")

        # transposed histories: hT0 = h2_0^T; hT1 = c2_1^T (+1 shifted);
        # tanhH = tanh(c2_1)^T; sTT = h2_1^T (built post-pass)
        hT = [xpp.tile([128, 4 * R], BF16, name=f"hT{l}") for l in range(L)]
        tanhH = xpp.tile([128, 4 * R], BF16, name="tanhH")
        sTT = xpp.tile([128, 4 * R], BF16, name="sTT")

        # ---------------- Phase A inputs (packed, few DMAs) ----------------
        pa = ctx.enter_context(tc.tile_pool(name="pa", bufs=1))
        xtt = pa.tile([128, 4 * R], BF16, tag="xtt", name="xtt")
        nc.sync.dma_start(
            xtt[:].rearrange("p (k c) -> p k c", k=4),
            XT0.ap()[0:512].rearrange("(k p) c -> p k c", k=4))
        xt4 = pa.tile([1, R], BF16, tag="xt4", name="xt4")
        nc.sync.dma_start(xt4[:], XT0.ap()[512:513, :])

        # PE warm-up: dummy matmuls on the identity while DMAs land
        wps = psp.tile([128, 128], F32, tag="g0", name="g0", bufs=2)
        for w in range(48):
            nc.tensor.matmul(wps[:], identb[:], identb[:],
                             start=True, stop=True, skip_group_check=True)

        # prefetch pool: recurrence weights + attention operands
        pf = ctx.enter_context(tc.tile_pool(name="pf", bufs=1))
        cTb = pf.tile([128, 2 * 128], BF16, tag="cTb", name="cTb")
        nc.sync.dma_start(
            cTb[:].rearrange("p (l c) -> p l c", l=2),
            CELLSP.ap().rearrange("l p c -> p l c"))
        c2T0b = pf.tile([128, 2 * 128], BF16, tag="c2T0b", name="c2T0b")
        nc.sync.dma_start(
            c2T0b[:].rearrange("p (l c) -> p l c", l=2),
            C2T0.ap().rearrange("l p c -> p l c"))
        wkt = pa.tile([128, 4 * G], BF16, tag="wkt", name="wkt")
        for k in range(4):
            nc.sync.dma_start(
                wkt[:, 2048 * k:2048 * (k + 1)],
                WIH0.ap()[128 * k:128 * (k + 1), :])
        wk4 = pa.tile([1, G], BF16, tag="wk4", name="wk4")
        nc.sync.dma_start(wk4[:], WIH0.ap()[512:513, :])
        whht = []
        for l, Wd in ((0, WHH0), (1, WHH1)):
            wt = pf.tile([128, 4 * G], BF16, tag=f"whh{l}", name=f"whh{l}")
            for k in range(4):
                nc.sync.dma_start(
                    wt[:, 2048 * k:2048 * (k + 1)],
                    Wd.ap()[128 * k:128 * (k + 1), :])
            whht.append(wt)
        wihC = pf.tile([128, 4 * G], BF16, tag="wihC", name="wihC")
        nc.sync.dma_start(
            wihC[:].rearrange("p (k c) -> p k c", k=4),
            WIH1.ap()[0:512].rearrange("(k p) c -> p k c", k=4))
        wihC4 = pf.tile([4, 512], BF16, tag="wihC4", name="wihC4")
        nc.sync.dma_start(wihC4[:], WIH1.ap()[512:513, :].rearrange(
            "a (j c) -> (a j) c", j=4))
        # ---------------- Phase A: xpart0 ----------------
        for mc in range(2):
            for nb in range(4):
                ps = gtile(nb % 2, [128, 512])
                for k in range(4):
                    nc.tensor.matmul(
                        ps[:],
                        xtt[:, 256 * k + 128 * mc:256 * k + 128 * (mc + 1)],
                        wkt[:, 2048 * k + 512 * nb:2048 * k + 512 * (nb + 1)],
                        start=(k == 0), stop=False)
                nc.tensor.matmul(
                    ps[:], xt4[:, 128 * mc:128 * (mc + 1)],
                    wk4[:, 512 * nb:512 * (nb + 1)],
                    start=False, stop=True)
                sb = pa.tile([128, 512], BF16, tag=f"stg{nb}", name=f"stg{nb}")
                nc.vector.tensor_copy(sb[:], ps[:])
                p0 = (4 * nb + 2 * mc) * 8
                dst = XPsb0[p0:p0 + 16, :].rearrange(
                    "p (q c) -> p q c", c=512)
                nc.sync.dma_start(dst, sb[:])

        # ---------------- Interleaved recurrence passes ----------------
        rp = ctx.enter_context(tc.tile_pool(name="rp", bufs=2))

        def linit(l):
            return {"l": l, "cT": cTb[:, 128 * l:128 * (l + 1)],
                    "c2T": c2T0b[:, 128 * l:128 * (l + 1)], "whh": whht[l],
                    "gates_pp": None, "c2h_prev": None, "tprev": -1}

        def lstep_mm(st, t):
            """Inject/xpart + W_hh rounds for step t (PE bulk)."""
            l = st["l"]
            gates = gtile(l, [128, 512 if l == 0 else 384])
            if l == 0:
                rhs = XPsb0[:, 512 * (t // 8):512 * (t // 8 + 1)]
                nc.tensor.matmul(
                    gates[:], einj8[:, 128 * (t % 8):128 * (t % 8 + 1)], rhs,
                    start=True, stop=False, skip_group_check=True)
            else:
                # xpart1(t) accumulated in place: bias row first (writes all
                # 128 partitions), then W_ih1 rounds from hT0 columns of t
                nc.tensor.matmul(gates[:], e4blk[:], wihC4[:, 0:384],
                                 start=True, stop=False, skip_group_check=True)
                for k in range(4):
                    lhsT = hT[0][:].rearrange(
                        "p (k b t) -> p k b t", k=4, b=BS)[:, k, :, t]
                    for j in range(4):
                        nc.tensor.matmul(
                            gates[32 * j:32 * j + BS, :],
                            lhsT,
                            wihC[:, 2048 * k + 512 * j:2048 * k + 512 * j + 384],
                            start=False, stop=False,
                            tile_position=(0, 32 * j), skip_group_check=True)
            w = 512 if l == 0 else 384
            for k in range(4):
                lhsT = st["c2T"][:, 32 * k:32 * k + BS]
                for j in range(4):
                    nc.tensor.matmul(
                        gates[32 * j:32 * j + BS, :],
                        lhsT,
                        st["whh"][:, 2048 * k + 512 * j:2048 * k + 512 * j + w],
                        start=False, stop=(k == 3),
                        tile_position=(0, 32 * j), skip_group_check=True)
            st["gates"] = gates

        def lstep_sigma(st, t):
            """sigma + c2 elementwise chain for step t (no transpose)."""
            l = st["l"]
            gates = st["gates"]
            sall = rp.tile([128, 384], F32, tag=f"sa{l}", name=f"sa{l}",
                           bufs=3)
            nc.scalar.activation(sall[:], gates[:, 0:384], AF.Sigmoid)
            m1 = rp.tile([128, 128], BF16, tag=f"m1{l}", name=f"m1{l}")
            nc.gpsimd.tensor_mul(m1[:], sall[:, 128:256], st["cT"])
            tgv = rp.tile([128, 128], BF16, tag=f"tg{l}", name=f"tg{l}")
            nc.vector.scalar_tensor_tensor(
                tgv[:], sall[:, 256:384], 2.0, ones128[:],
                mybir.AluOpType.mult, mybir.AluOpType.subtract)
            m2 = rp.tile([128, 128], BF16, tag=f"m2{l}", name=f"m2{l}")
            nc.vector.tensor_mul(m2[:], sall[:, 0:128], tgv[:])
            c2h = rp.tile([128, 128], BF16, tag=f"c2h{l}", name=f"c2h{l}",
                          bufs=3)
            nc.vector.tensor_add(c2h[:], m1[:], m2[:])
            st["sall"] = sall
            st["c2h"] = c2h
            st["gates_prev"] = st["gates"]

        def lstep_transpose(st, t):
            """c2 transpose + cast — emitted at a PE-queue position where
            the chain dependency (add) has already resolved."""
            l = st["l"]
            tp = psp.tile([128, 128], BF16, tag=f"tp{l}", name=f"tp{l}",
                          bufs=2)
            nc.tensor.transpose(tp[:], st["c2h"][:], identb[:])
            c2T_new = rp.tile([128, 128], BF16, tag=f"c2T{l}", name=f"c2T{l}")
            nc.vector.tensor_copy(c2T_new[:], tp[:])
            st["c2T"] = c2T_new
            if l == 1:
                # histories for the deferred o-gate / h2_1 reconstruction
                tview = tp[:].rearrange("p (k r) -> p k r", k=4)[:, :, 0:BS]
                if t < T - 1:
                    dstc = hT[1][:].rearrange(
                        "p (k b t) -> p k b t", k=4, b=BS)[:, :, :, t + 1]
                    nc.vector.tensor_copy(dstc, tview)
                th = rp.tile([128, 128], BF16, tag="th1", name="th1")
                nc.scalar.activation(th[:], tp[:], AF.Tanh)
                dstt = tanhH[:].rearrange(
                    "p (k b t) -> p k b t", k=4, b=BS)[:, :, :, t]
                nc.vector.tensor_copy(
                    dstt, th[:].rearrange("p (k r) -> p k r", k=4)[:, :, 0:BS])

        def lstep_branch(st):
            """Delayed h2 branch for the PREVIOUS step (never blocks chains)."""
            l = st["l"]
            if l == 1:
                return
            if st["c2h_prev"] is not None:
                so = rp.tile([128, 128], BF16, tag=f"so{l}", name=f"so{l}")
                nc.scalar.activation(so[:], st["gates_pp"][:, 384:512],
                                     AF.Sigmoid)
                tc2 = rp.tile([128, 128], BF16, tag=f"tc2{l}", name=f"tc2{l}")
                nc.scalar.activation(tc2[:], st["c2h_prev"][:], AF.Tanh)
                h2 = rp.tile([128, 128], BF16, tag=f"h2{l}", name=f"h2{l}")
                nc.gpsimd.tensor_mul(h2[:], so[:], tc2[:])
                tp2 = psp.tile([128, 128], BF16, tag=f"tp{l}", name=f"tp{l}",
                               bufs=2)
                nc.tensor.transpose(tp2[:], h2[:], identb[:])
                src_ = tp2[:].rearrange("p (k r) -> p k r", k=4)[:, :, 0:BS]
                dst = hT[l][:].rearrange(
                    "p (k b t) -> p k b t", k=4, b=BS)[:, :, :, st["tprev"]]
                nc.vector.tensor_copy(dst, src_)
            st["c2h_prev"] = st["c2h"]
            st["gates_pp"] = st["gates_prev"]
            st["tprev"] = st["tprev"] + 1

        # phase-E operands: transferred during the recurrence pass
        # phase-E operands: transferred during the recurrence pass
        wiho = pf.tile([128, 4 * 512], BF16, tag="wiho", name="wiho")
        nc.sync.dma_start(
            wiho[:].rearrange("p (k c) -> p k c", k=4),
            WIHO.ap()[0:512].rearrange("(k p) c -> p k c", k=4))
        wihob = pf.tile([1, 512], BF16, tag="wihob", name="wihob")
        nc.sync.dma_start(wihob[:], WIHO.ap()[512:513, :])
        whho = pf.tile([128, 4 * 512], BF16, tag="whho", name="whho")
        nc.sync.dma_start(
            whho[:].rearrange("p (k c) -> p k c", k=4),
            WHHO.ap().rearrange("(k p) c -> p k c", k=4))
        wint = pf.tile([128, 4 * DS], BF16, tag="wint", name="wint")
        nc.sync.dma_start(
            wint[:].rearrange("p (k c) -> p k c", k=4),
            WINT.ap().rearrange("(k p) c -> p k c", k=4))
        encb = pf.tile([S, 4 * DS], BF16, tag="encb", name="encb")
        nc.sync.dma_start(
            encb[:].rearrange("p (b c) -> p b c", b=BS),
            ENC.ap().rearrange("b s d -> s b d"))
        enctb = pf.tile([128, BS * 8 * S], BF16, tag="enctb", name="enctb")
        nc.sync.dma_start(
            enctb[:].rearrange("p (b k s) -> p b k s", b=BS, k=8),
            ENCT.ap().rearrange("b (k p) s -> p b k s", k=8))
        offsb = pf.tile([T, BS * S], F32, tag="offsb", name="offsb")
        nc.sync.dma_start(
            offsb[:].rearrange("p (b s) -> p b s", b=BS),
            OFFS.ap().rearrange("b t s -> t b s"))
        woutt = pf.tile([128, 12 * D], BF16, tag="woutt", name="woutt")
        nc.sync.dma_start(
            woutt[:].rearrange("p (k c) -> p k c", k=12),
            WOUTT.ap()[0:1536].rearrange("(k p) c -> p k c", k=12))
        woutb = pf.tile([1, D], BF16, tag="woutb", name="woutb")
        nc.sync.dma_start(woutb[:], WOUTT.ap()[1536:1537, :])

        st0 = linit(0)
        st1 = linit(1)
        # hT1 col 0 = initial c2_1
        nc.vector.tensor_copy(
            hT[1][:].rearrange("p (k b t) -> p k b t", k=4, b=BS)[:, :, :, 0],
            c2T0b[:, 128:256].rearrange("p (k r) -> p k r", k=4)[:, :, 0:BS])
        for ss in range(T + LAG + 1):
            if ss < T:
                lstep_mm(st0, ss)                # PE: inj + rounds L0(t)
            if LAG < ss <= T + LAG:
                lstep_transpose(st1, ss - LAG - 1)  # PE: T_c2 L1(t'-1)
            if ss < T:
                lstep_sigma(st0, ss)
            if LAG <= ss < T + LAG:
                lstep_mm(st1, ss - LAG)          # PE: bias/xpart/whh L1(t')
            if LAG <= ss < T + LAG:
                lstep_sigma(st1, ss - LAG)
            if ss < T:
                lstep_transpose(st0, ss)         # PE: T_c2 L0(t) (ready)
            if ss <= T:
                lstep_branch(st0)                # PE: T_h2 L0(t-1)

        # ---------------- deferred o-gate / h2_1 reconstruction ----------
        with tc.tile_pool(name="po", bufs=1) as po:
            for mc in range(2):
                msl = slice(128 * mc, 128 * (mc + 1))
                ps = gtile(mc, [128, 512])
                for k in range(4):
                    nc.tensor.matmul(
                        ps[:], hT[0][:, 256 * k + 128 * mc:
                                     256 * k + 128 * (mc + 1)],
                        wiho[:, 512 * k:512 * (k + 1)],
                        start=(k == 0), stop=False)
                for k in range(4):
                    nc.tensor.matmul(
                        ps[:], hT[1][:, 256 * k + 128 * mc:
                                     256 * k + 128 * (mc + 1)],
                        whho[:, 512 * k:512 * (k + 1)],
                        start=False, stop=False)
                nc.tensor.matmul(
                    ps[:], ones[0:1, 0:128], wihob[:],
                    start=False, stop=True)
                sob = po.tile([128, 512], BF16, tag=f"sob{mc}",
                              name=f"sob{mc}")
                nc.scalar.activation(sob[:], ps[:], AF.Sigmoid)
                for j in range(4):
                    tpo = psp.tile([128, 128], BF16, tag=f"tp{j % 2}",
                                   name=f"tp{j % 2}", bufs=2)
                    nc.tensor.transpose(
                        tpo[:], sob[:, 128 * j:128 * (j + 1)], identb[:])
                    dsl = slice(256 * j + 128 * mc, 256 * j + 128 * (mc + 1))
                    nc.vector.tensor_mul(
                        sTT[:, dsl], tpo[:], tanhH[:, dsl])

        # ---------------- Phase E: attention + out proj ----------------
        wkt2 = pa.tile([128, 4 * G], BF16, tag="wkt", name="wkt2")
        with tc.tile_pool(name="pe", bufs=1) as pe:
            sT = [sTT[:, 256 * k:256 * (k + 1)] for k in range(4)]

            xqT = []
            for m in range(8):
                ps = gtile(m % 2, [128, R])
                for k in range(4):
                    nc.tensor.matmul(
                        ps[:], wint[:, 1024 * k + 128 * m:1024 * k + 128 * (m + 1)],
                        sT[k], start=(k == 0), stop=(k == 3))
                xq = wkt2[:, 256 * m:256 * (m + 1)]
                if m % 2 == 0:
                    nc.scalar.copy(xq, ps[:])
                else:
                    nc.vector.tensor_copy(xq, ps[:])
                xqT.append(xq)

            ctxT = [wkt2[:, 2048 + 256 * m:2048 + 256 * (m + 1)]
                    for m in range(8)]
            for b in range(BS):
                bsl = slice(T * b, T * (b + 1))
                eps = psp.tile([T, S], F32, tag=f"g{b % 2}",
                               name=f"g{b % 2}", bufs=2)
                for k in range(8):
                    nc.tensor.matmul(
                        eps[:], xqT[k][:, bsl],
                        enctb[:, 1024 * b + 128 * k:1024 * b + 128 * (k + 1)],
                        start=(k == 0), stop=(k == 7))
                esb = pe.tile([T, S], F32, tag=f"esb{b % 2}",
                              name=f"esb{b % 2}")
                nc.vector.tensor_add(esb[:], eps[:],
                                     offsb[:, 128 * b:128 * (b + 1)])
                negmax = pe.tile([T, 1], F32, tag=f"negmax{b % 2}",
                                 name=f"negmax{b % 2}")
                nc.vector.reduce_max(
                    negmax[:], esb[:], axis=mybir.AxisListType.X, negate=True)
                expE = pe.tile([T, S], F32, tag=f"expE{b % 2}",
                               name=f"expE{b % 2}")
                den = pe.tile([T, 1], F32, tag=f"den{b % 2}",
                              name=f"den{b % 2}")
                nc.scalar.activation(
                    expE[:], esb[:], AF.Exp, bias=negmax[:], accum_out=den[:])
                rden = pe.tile([T, 1], F32, tag=f"rden{b % 2}",
                               name=f"rden{b % 2}")
                nc.vector.reciprocal(rden[:], den[:])
                attn = pe.tile([T, S], F32, tag=f"attn{b % 2}",
                               name=f"attn{b % 2}")
                nc.vector.tensor_scalar_mul(attn[:], expE[:], rden[:])
                tpa = psp.tile([S, T], F32, tag=f"tp{b % 2}",
                               name=f"tp{b % 2}", bufs=2)
                nc.tensor.transpose(tpa[:], attn[:], ident[0:T, 0:T])
                atsb = pe.tile([S, T], BF16, tag=f"atsb{b % 2}",
                               name=f"atsb{b % 2}")
                nc.vector.tensor_copy(atsb[:], tpa[:])
                for m in range(8):
                    psc = gtile(m % 2, [128, T])
                    nc.tensor.matmul(
                        psc[:], encb[:, 1024 * b + 128 * m:1024 * b + 128 * (m + 1)],
                        atsb[:], start=True, stop=True)
                    if m % 2 == 0:
                        nc.scalar.copy(ctxT[m][:, bsl], psc[:])
                    else:
                        nc.vector.tensor_copy(ctxT[m][:, bsl], psc[:])

            outflat = OUT.ap().rearrange("b t d -> (b t) d")
            lhs_all = ctxT + sT + [ones]
            wt_all = [woutt[:, 512 * k:512 * (k + 1)] for k in range(12)] \
                + [woutb[:]]
            for mc in range(2):
                msl = slice(128 * mc, 128 * (mc + 1))
                ps = gtile(mc, [128, D])
                for k in range(13):
                    nc.tensor.matmul(
                        ps[:], lhs_all[k][:, msl], wt_all[k],
                        start=(k == 0), stop=(k == 12))
                osb = pa.tile([128, D], F32, tag=f"stg{mc}", name=f"osb{mc}")
                nc.scalar.activation(osb[:], ps[:], AF.Tanh)
                nc.sync.dma_start(outflat[msl, :], osb[:])

    nc.compile()
    return nc


def assemble(results):
    full = np.concatenate([r["out"] for r in results], axis=0)  # [B, T, D]
    outs = full.transpose(1, 0, 2)                              # [T, B, D]
    return np.ascontiguousarray(outs.reshape(-1, D).reshape(-1, T, D))


_nc_cache = None


def kernel(**inputs):
    global _nc_cache
    in_maps = host_prep(inputs)
    if _nc_cache is None:
        _nc_cache = build_program()
    res = run_bass_kernel_spmd(_nc_cache, in_maps, list(range(NCORES)))
    return assemble(res.results)


# revision 28
# speedup vs baseline: 1.1570x; 1.0427x over previous
"""Trainium2 Bass kernel for nn_Decoder_46042049413334.

Buggy 2-layer LSTM decoder with attention (B=32, T=64, S=128, D=512).

Structure (per core, batch sharded 8 ways, BS=4 examples/core):
  Phase A: xpart0 = [emb(tokens), 1] @ [W_ih0.T; b0]  -> XPsb0 (SBUF)
  Interleaved pass: layer-0 step t and layer-1 step t-2 run together;
    layer-1's xpart is accumulated per step directly into its gates
    PSUM from the transposed h2 history (hT0), so the PE stays busy
    enough to hold the HAM clock gate open (K=8/8).
  Phase E: attention + out-projection from hT1

Recurrence layout: gates PSUM [128, 512] where partition 32*j+b holds
(example b, d-block j) and the 512 free cols are {i,f,o,2g}x128 for
that d-block (g columns pre-scaled by 2 so tanh(g) = 2*sigmoid(2g)-1
comes out of a single full-width sigmoid). The four d-blocks' weight
streams run CONCURRENTLY in the PE array via tile_position=(0, 32*j)
column tiling. Elementwise runs once over all 128 partitions; c2 and
h2 are re-transposed per step ([128,128] PE transpose). Off-chain work
(tanh(c2), h2, its transpose/gather) is emitted one step late so no
engine FIFO ever blocks the recurrence chains.

Row ordering is b-major everywhere: row r = b_local*T + t.
"""
import numpy as np
import ml_dtypes
from contextlib import ExitStack

import concourse.bass as bass
import concourse.bacc as bacc
import concourse.tile as tile
from concourse import mybir, masks
from concourse.bass_utils import run_bass_kernel_spmd

F32 = mybir.dt.float32
BF16 = mybir.dt.bfloat16
AF = mybir.ActivationFunctionType
NPBF = ml_dtypes.bfloat16

B, T, S, D, L, V = 32, 64, 128, 512, 2, 32000
G = 4 * D        # 2048
DS = 2 * D       # 1024
NCORES = 8
BS = B // NCORES  # 4
R = BS * T        # 256 rows per core
LAG = 2          # layer-1 recurrence lag behind layer 0


# ---------------------------------------------------------------- host side

def _gate_perm():
    perm = np.zeros(G, dtype=np.int64)
    base = {0: 0, 1: 512, 2: 1536, 3: 1024}  # i, f, o, g
    for j in range(G):
        nb, pos = divmod(j, 512)
        sub, dd = divmod(pos, 128)
        perm[j] = base[sub] + nb * 128 + dd
    return perm


def host_prep(inputs):
    """Build the 8 per-core input maps (layout/gather work only)."""
    perm = _gate_perm()
    tokens = np.asarray(inputs["prev_tgt_tokens"])
    embed = np.asarray(inputs["embed"], dtype=np.float32)
    enc = np.asarray(inputs["encoder_out"], dtype=np.float32)
    mask = np.asarray(inputs["src_mask"])
    hid = np.asarray(inputs["hiddens"], dtype=np.float32)
    cells = np.asarray(inputs["cells"], dtype=np.float32)
    W_ih = np.asarray(inputs["W_ih"], dtype=np.float32)
    W_hh = np.asarray(inputs["W_hh"], dtype=np.float32)
    b_ih = np.asarray(inputs["b_ih"], dtype=np.float32)
    b_hh = np.asarray(inputs["b_hh"], dtype=np.float32)
    W_in = np.asarray(inputs["W_in"], dtype=np.float32)
    b_in = np.asarray(inputs["b_in"], dtype=np.float32)
    W_out = np.asarray(inputs["W_out"], dtype=np.float32)
    b_out = np.asarray(inputs["b_out"], dtype=np.float32)

    def bf(x):
        return np.ascontiguousarray(x, dtype=NPBF)

    # layer 0 keeps sub-order [i,f,o,2g]; layer 1 uses [i,f,2g,o] so the
    # in-loop stream is the contiguous 384 cols {i,f,2g} and o defers.
    perm1 = np.zeros(G, dtype=np.int64)
    base1 = {0: 0, 1: 512, 2: 1024, 3: 1536}  # i, f, g, o
    for jj in range(G):
        nb_, pos = divmod(jj, 512)
        sub, dd = divmod(pos, 128)
        perm1[jj] = base1[sub] + nb_ * 128 + dd
    WIH = []
    WHH = []
    gscale = np.ones(G, np.float32)
    gscale1 = np.ones(G, np.float32)
    for nb in range(4):
        gscale[512 * nb + 384:512 * (nb + 1)] = 2.0   # tanh(g)=2*sig(2g)-1
        gscale1[512 * nb + 256:512 * nb + 384] = 2.0
    for l, pm, gs in ((0, perm1, gscale1), (1, perm1, gscale1)):
        wihT = W_ih[l].T[:, pm] * gs
        biasrow = ((b_ih[l] + b_hh[l])[pm] * gs)[None, :]
        WIH.append(bf(np.concatenate([wihT, biasrow], 0)))   # [513, 2048]
        WHH.append(bf(W_hh[l].T[:, pm] * gs))                # [512, 2048]
    # layer-1 o-gate weights, block-major od = j*128+d
    osel = np.concatenate([np.arange(1536 + 128 * j, 1536 + 128 * (j + 1))
                           for j in range(4)])
    WIHO = bf(np.concatenate(
        [W_ih[1].T[:, osel],
         (b_ih[1] + b_hh[1])[osel][None, :]], 0))            # [513, 512]
    WHHO = bf(W_hh[1].T[:, osel])                            # [512, 512]
    WINT = bf(W_in.T)                                        # [512, 1024]
    WOUTT = bf(np.concatenate([W_out.T, b_out[None, :]], 0))  # [1537, 512]

    # xpart0 injection selectors, one per u = t%8:
    # XPsb0 partition (4j+b)*8+u feeds gates row 32j+b
    einj8 = np.zeros((8, 128, 128), np.float32)
    for u in range(8):
        for j in range(4):
            for b in range(BS):
                einj8[u, (4 * j + b) * 8 + u, 32 * j + b] = 1.0
    einj8 = bf(einj8.transpose(1, 0, 2).reshape(128, 8 * 128))

    # block selector: e4blk[j, p] = 1 iff p // 32 == j
    e4 = np.zeros((4, 128), np.float32)
    for j in range(4):
        e4[j, 32 * j:32 * (j + 1)] = 1.0
    e4 = bf(e4)

    in_maps = []
    for core in range(NCORES):
        bsl = slice(core * BS, (core + 1) * BS)
        xe = embed[tokens[bsl]]                              # [BS, T, D]
        Xaug = np.concatenate(
            [xe.reshape(R, D), np.ones((R, 1), np.float32)], axis=1)
        # permute rows so phase-A store DMAs land partition-parallel:
        # new row b*64 + (t%8)*8 + t//8  <- (b, t)
        rperm = np.zeros(R, np.int64)
        for b in range(BS):
            for t in range(T):
                rperm[b * T + (t % 8) * 8 + t // 8] = b * T + t
        XT0 = bf(Xaug[rperm].T)                              # [513, 256]
        enc_c = np.ascontiguousarray(enc[bsl])               # [BS, 128, 1024]
        encT_c = np.swapaxes(enc_c, 1, 2)                    # [BS, 1024, 128]
        offs = np.einsum("bsd,d->bs", enc_c, b_in) + np.where(mask[bsl], -1e9, 0.0)
        offs_rep = np.ascontiguousarray(
            np.broadcast_to(offs[:, None, :], (BS, T, S)), dtype=np.float32)
        # initial c2T: c2t0[l, p, 32k+b] = hid[l, b, 128k+p]
        th = hid[:, bsl].reshape(L, BS, 4, 128).transpose(0, 3, 2, 1)  # [L,128,4,BS]
        c2t0 = np.zeros((L, 128, 4, 32), np.float32)
        c2t0[:, :, :, 0:BS] = th
        c2t0 = bf(c2t0.reshape(L, 128, 128))
        # cells in partition layout: cellsp[l, 32j+b, p] = cells[l, b, 128j+p]
        tc_ = cells[:, bsl].reshape(L, BS, 4, 128).transpose(0, 2, 1, 3)  # [L,4,BS,128]
        cellsp = np.zeros((L, 4, 32, 128), np.float32)
        cellsp[:, :, 0:BS, :] = tc_
        cellsp = bf(cellsp.reshape(L, 128, 128))
        in_maps.append({
            "xt0": XT0,
            "wih0": WIH[0], "whh0": WHH[0],
            "wih1": WIH[1], "whh1": WHH[1],
            "wiho": WIHO, "whho": WHHO,
            "wint": WINT, "woutt": WOUTT,
            "enc": bf(enc_c), "enct": bf(encT_c), "offs": offs_rep,
            "c2t0": c2t0, "cellsp": cellsp,
            "ones1": np.ones((1, R), NPBF),
            "einj8": einj8, "e4blk": e4,
        })
    return in_maps


# ------------------------------------------------------------- device build

def build_program():
    nc = bacc.Bacc("TRN2", target_bir_lowering=False, debug=False)

    XT0 = nc.dram_tensor("xt0", [513, R], BF16, kind="ExternalInput")
    WIH0 = nc.dram_tensor("wih0", [513, G], BF16, kind="ExternalInput")
    WHH0 = nc.dram_tensor("whh0", [D, G], BF16, kind="ExternalInput")
    WIH1 = nc.dram_tensor("wih1", [513, G], BF16, kind="ExternalInput")
    WHH1 = nc.dram_tensor("whh1", [D, G], BF16, kind="ExternalInput")
    WIHO = nc.dram_tensor("wiho", [513, 512], BF16, kind="ExternalInput")
    WHHO = nc.dram_tensor("whho", [D, 512], BF16, kind="ExternalInput")
    WINT = nc.dram_tensor("wint", [D, DS], BF16, kind="ExternalInput")
    WOUTT = nc.dram_tensor("woutt", [DS + D + 1, D], BF16, kind="ExternalInput")
    ENC = nc.dram_tensor("enc", [BS, S, DS], BF16, kind="ExternalInput")
    ENCT = nc.dram_tensor("enct", [BS, DS, S], BF16, kind="ExternalInput")
    OFFS = nc.dram_tensor("offs", [BS, T, S], F32, kind="ExternalInput")
    C2T0 = nc.dram_tensor("c2t0", [L, 128, 128], BF16, kind="ExternalInput")
    CELLSP = nc.dram_tensor("cellsp", [L, 128, 128], BF16, kind="ExternalInput")
    ONES1 = nc.dram_tensor("ones1", [1, R], BF16, kind="ExternalInput")
    EINJ = nc.dram_tensor("einj8", [128, 8 * 128], BF16, kind="ExternalInput")
    E4BLK = nc.dram_tensor("e4blk", [4, 128], BF16, kind="ExternalInput")
    OUT = nc.dram_tensor("out", [BS, T, D], F32, kind="ExternalOutput")

    with tile.TileContext(nc) as tc, ExitStack() as ctx:
        cpool = ctx.enter_context(tc.tile_pool(name="const", bufs=1))
        ident = cpool.tile([128, 128], F32)
        masks.make_identity(nc, ident[:])
        identb = cpool.tile([128, 128], BF16, name="identb")
        masks.make_identity(nc, identb[:])
        ones = cpool.tile([1, R], BF16, name="ones")
        nc.sync.dma_start(ones[:], ONES1.ap())
        einj8 = cpool.tile([128, 8 * 128], BF16, name="einj8")
        nc.sync.dma_start(einj8[:], EINJ.ap())
        e4blk = cpool.tile([4, 128], BF16, name="e4blk")
        nc.sync.dma_start(e4blk[:], E4BLK.ap())
        ones128 = cpool.tile([128, 128], BF16, name="ones128")
        nc.gpsimd.memset(ones128[:], 1.0)
        tw = cpool.tile([1, 4], F32, name="tw")
        nc.scalar.activation(tw[:], ones128[0:1, 0:4], AF.Sigmoid)
        nc.scalar.activation(tw[:], ones128[0:1, 0:4], AF.Tanh)
        nc.scalar.activation(tw[:], ones128[0:1, 0:4], AF.Exp)

        psp = ctx.enter_context(tc.tile_pool(name="ps", bufs=1, space="PSUM"))

        def gtile(idx, shape):
            return psp.tile(shape, F32, tag=f"g{idx}", name=f"g{idx}",
                            bufs=2 if idx < 2 else 1)

        # persistent SBUF xpart0:
        # XPsb0[(4*nb+b)*8 + t%8, (t//8)*512 + c] = xpart0[b,t,512nb+c]
        xpp = ctx.enter_context(tc.tile_pool(name="xps", bufs=1))
        XPsb0 = xpp.tile([128, (T // 8) * 512], BF16, name="xpsb0")

        # transposed histories: hT0 = h2_0^T; hT1 = c2_1^T (+1 shifted);
        # tanhH = tanh(c2_1)^T; sTT = h2_1^T (built post-pass)
        hT = [xpp.tile([128, 4 * R], BF16, name=f"hT{l}") for l in range(L)]
        tanhH = xpp.tile([128, 4 * R], BF16, name="tanhH")
        sTT = xpp.tile([128, 4 * R], BF16, name="sTT")

        # ---------------- Phase A inputs (packed, few DMAs) ----------------
        pa = ctx.enter_context(tc.tile_pool(name="pa", bufs=1))
        xtt = pa.tile([128, 4 * R], BF16, tag="xtt", name="xtt")
        nc.sync.dma_start(
            xtt[:].rearrange("p (k c) -> p k c", k=4),
            XT0.ap()[0:512].rearrange("(k p) c -> p k c", k=4))
        xt4 = pa.tile([1, R], BF16, tag="xt4", name="xt4")
        nc.sync.dma_start(xt4[:], XT0.ap()[512:513, :])

        # PE warm-up: dummy matmuls on the identity while DMAs land
        wps = psp.tile([128, 128], F32, tag="g0", name="g0", bufs=2)
        for w in range(48):
            nc.tensor.matmul(wps[:], identb[:], identb[:],
                             start=True, stop=True, skip_group_check=True)

        # prefetch pool: recurrence weights + attention operands
        pf = ctx.enter_context(tc.tile_pool(name="pf", bufs=1))
        cTb = pf.tile([128, 2 * 128], BF16, tag="cTb", name="cTb")
        nc.sync.dma_start(
            cTb[:].rearrange("p (l c) -> p l c", l=2),
            CELLSP.ap().rearrange("l p c -> p l c"))
        c2T0b = pf.tile([128, 2 * 128], BF16, tag="c2T0b", name="c2T0b")
        nc.sync.dma_start(
            c2T0b[:].rearrange("p (l c) -> p l c", l=2),
            C2T0.ap().rearrange("l p c -> p l c"))
        wkt = pa.tile([128, 4 * G], BF16, tag="wkt", name="wkt")
        for k in range(4):
            nc.sync.dma_start(
                wkt[:, 2048 * k:2048 * (k + 1)],
                WIH0.ap()[128 * k:128 * (k + 1), :])
        wk4 = pa.tile([1, G], BF16, tag="wk4", name="wk4")
        nc.sync.dma_start(wk4[:], WIH0.ap()[512:513, :])
        whht = []
        for l, Wd in ((0, WHH0), (1, WHH1)):
            wt = pf.tile([128, 4 * G], BF16, tag=f"whh{l}", name=f"whh{l}")
            for k in range(4):
                nc.sync.dma_start(
                    wt[:, 2048 * k:2048 * (k + 1)],
                    Wd.ap()[128 * k:128 * (k + 1), :])
            whht.append(wt)
        wihC = pf.tile([128, 4 * G], BF16, tag="wihC", name="wihC")
        nc.sync.dma_start(
            wihC[:].rearrange("p (k c) -> p k c", k=4),
            WIH1.ap()[0:512].rearrange("(k p) c -> p k c", k=4))
        wihC4 = pf.tile([4, 512], BF16, tag="wihC4", name="wihC4")
        nc.sync.dma_start(wihC4[:], WIH1.ap()[512:513, :].rearrange(
            "a (j c) -> (a j) c", j=4))
        # ---------------- Phase A: xpart0 ----------------
        for mc in range(2):
            for nb in range(4):
                ps = gtile(nb % 2, [128, 512])
                for k in range(4):
                    nc.tensor.matmul(
                        ps[:],
                        xtt[:, 256 * k + 128 * mc:256 * k + 128 * (mc + 1)],
                        wkt[:, 2048 * k + 512 * nb:2048 * k + 512 * (nb + 1)],
                        start=(k == 0), stop=False)
                nc.tensor.matmul(
                    ps[:], xt4[:, 128 * mc:128 * (mc + 1)],
                    wk4[:, 512 * nb:512 * (nb + 1)],
                    start=False, stop=True)
                sb = pa.tile([128, 512], BF16, tag=f"stg{nb}", name=f"stg{nb}")
                nc.vector.tensor_copy(sb[:], ps[:])
                p0 = (4 * nb + 2 * mc) * 8
                dst = XPsb0[p0:p0 + 16, :].rearrange(
                    "p (q c) -> p q c", c=512)
                nc.sync.dma_start(dst, sb[:])

        # ---------------- Interleaved recurrence passes ----------------
        rp = ctx.enter_context(tc.tile_pool(name="rp", bufs=2))

        def linit(l):
            return {"l": l, "cT": cTb[:, 128 * l:128 * (l + 1)],
                    "c2T": c2T0b[:, 128 * l:128 * (l + 1)], "whh": whht[l],
                    "gates_pp": None, "c2h_prev": None, "tprev": -1}

        def lstep_mm(st, t):
            """Inject/xpart + W_hh rounds for step t (PE bulk)."""
            l = st["l"]
            gates = gtile(l, [128, 512 if l == 0 else 384])
            if l == 0:
                rhs = XPsb0[:, 512 * (t // 8):512 * (t // 8 + 1)]
                nc.tensor.matmul(
                    gates[:], einj8[:, 128 * (t % 8):128 * (t % 8 + 1)], rhs,
                    start=True, stop=False, skip_group_check=True)
            else:
                # xpart1(t) accumulated in place: bias row first (writes all
                # 128 partitions), then W_ih1 rounds from hT0 columns of t
                nc.tensor.matmul(gates[:], e4blk[:], wihC4[:, 0:384],
                                 start=True, stop=False, skip_group_check=True)
                for k in range(4):
                    lhsT = hT[0][:].rearrange(
                        "p (k b t) -> p k b t", k=4, b=BS)[:, k, :, t]
                    for j in range(4):
                        nc.tensor.matmul(
                            gates[32 * j:32 * j + BS, :],
                            lhsT,
                            wihC[:, 2048 * k + 512 * j:2048 * k + 512 * j + 384],
                            start=False, stop=False,
                            tile_position=(0, 32 * j), skip_group_check=True)
            w = 512 if l == 0 else 384
            for k in range(4):
                lhsT = st["c2T"][:, 32 * k:32 * k + BS]
                for j in range(4):
                    nc.tensor.matmul(
                        gates[32 * j:32 * j + BS, :],
                        lhsT,
                        st["whh"][:, 2048 * k + 512 * j:2048 * k + 512 * j + w],
                        start=False, stop=(k == 3),
                        tile_position=(0, 32 * j), skip_group_check=True)
            st["gates"] = gates

        def lstep_sigma(st, t):
            """sigma + c2 elementwise chain for step t (no transpose)."""
            l = st["l"]
            gates = st["gates"]
            sall = rp.tile([128, 384], F32, tag=f"sa{l}", name=f"sa{l}",
                           bufs=3)
            nc.scalar.activation(sall[:], gates[:, 0:384], AF.Sigmoid)
            m1 = rp.tile([128, 128], BF16, tag=f"m1{l}", name=f"m1{l}")
            nc.gpsimd.tensor_mul(m1[:], sall[:, 128:256], st["cT"])
            # c2 = m1 + 2*[sig(i) * (sig(2g) - 0.5)]  (two fused DVE ops)
            m2h = rp.tile([128, 128], BF16, tag=f"m2{l}", name=f"m2{l}")
            nc.vector.scalar_tensor_tensor(
                m2h[:], sall[:, 256:384], 0.5, sall[:, 0:128],
                mybir.AluOpType.subtract, mybir.AluOpType.mult)
            c2h = rp.tile([128, 128], BF16, tag=f"c2h{l}", name=f"c2h{l}",
                          bufs=3)
            nc.vector.scalar_tensor_tensor(
                c2h[:], m2h[:], 2.0, m1[:],
                mybir.AluOpType.mult, mybir.AluOpType.add)
            st["sall"] = sall
            st["c2h"] = c2h
            st["gates_prev"] = st["gates"]

        def lstep_transpose(st, t):
            """c2 transpose + cast — emitted at a PE-queue position where
            the chain dependency (add) has already resolved."""
            l = st["l"]
            tp = psp.tile([128, 128], BF16, tag=f"tp{l}", name=f"tp{l}",
                          bufs=2)
            nc.tensor.transpose(tp[:], st["c2h"][:], identb[:])
            c2T_new = rp.tile([128, 128], BF16, tag=f"c2T{l}", name=f"c2T{l}")
            nc.vector.tensor_copy(c2T_new[:], tp[:])
            st["c2T"] = c2T_new
            if l == 1:
                # histories for the deferred o-gate / h2_1 reconstruction
                tview = tp[:].rearrange("p (k r) -> p k r", k=4)[:, :, 0:BS]
                if t < T - 1:
                    dstc = hT[1][:].rearrange(
                        "p (k b t) -> p k b t", k=4, b=BS)[:, :, :, t + 1]
                    nc.vector.tensor_copy(dstc, tview)
                th = rp.tile([128, 128], BF16, tag="th1", name="th1")
                nc.scalar.activation(th[:], tp[:], AF.Tanh)
                dstt = tanhH[:].rearrange(
                    "p (k b t) -> p k b t", k=4, b=BS)[:, :, :, t]
                nc.vector.tensor_copy(
                    dstt, th[:].rearrange("p (k r) -> p k r", k=4)[:, :, 0:BS])

        def lstep_branch(st):
            """Delayed h2 branch for the PREVIOUS step (never blocks chains)."""
            l = st["l"]
            if l == 1:
                return
            if st["c2h_prev"] is not None:
                so = rp.tile([128, 128], BF16, tag=f"so{l}", name=f"so{l}")
                nc.scalar.activation(so[:], st["gates_pp"][:, 384:512],
                                     AF.Sigmoid)
                tc2 = rp.tile([128, 128], BF16, tag=f"tc2{l}", name=f"tc2{l}")
                nc.scalar.activation(tc2[:], st["c2h_prev"][:], AF.Tanh)
                h2 = rp.tile([128, 128], BF16, tag=f"h2{l}", name=f"h2{l}")
                nc.gpsimd.tensor_mul(h2[:], so[:], tc2[:])
                tp2 = psp.tile([128, 128], BF16, tag=f"tp{l}", name=f"tp{l}",
                               bufs=2)
                nc.tensor.transpose(tp2[:], h2[:], identb[:])
                src_ = tp2[:].rearrange("p (k r) -> p k r", k=4)[:, :, 0:BS]
                dst = hT[l][:].rearrange(
                    "p (k b t) -> p k b t", k=4, b=BS)[:, :, :, st["tprev"]]
                nc.vector.tensor_copy(dst, src_)
            st["c2h_prev"] = st["c2h"]
            st["gates_pp"] = st["gates_prev"]
            st["tprev"] = st["tprev"] + 1

        # phase-E operands: transferred during the recurrence pass
        # phase-E operands: transferred during the recurrence pass
        wiho = pf.tile([128, 4 * 512], BF16, tag="wiho", name="wiho")
        nc.sync.dma_start(
            wiho[:].rearrange("p (k c) -> p k c", k=4),
            WIHO.ap()[0:512].rearrange("(k p) c -> p k c", k=4))
        wihob = pf.tile([1, 512], BF16, tag="wihob", name="wihob")
        nc.sync.dma_start(wihob[:], WIHO.ap()[512:513, :])
        whho = pf.tile([128, 4 * 512], BF16, tag="whho", name="whho")
        nc.sync.dma_start(
            whho[:].rearrange("p (k c) -> p k c", k=4),
            WHHO.ap().rearrange("(k p) c -> p k c", k=4))
        wint = pf.tile([128, 4 * DS], BF16, tag="wint", name="wint")
        nc.sync.dma_start(
            wint[:].rearrange("p (k c) -> p k c", k=4),
            WINT.ap().rearrange("(k p) c -> p k c", k=4))
        encb = pf.tile([S, 4 * DS], BF16, tag="encb", name="encb")
        nc.sync.dma_start(
            encb[:].rearrange("p (b c) -> p b c", b=BS),
            ENC.ap().rearrange("b s d -> s b d"))
        enctb = pf.tile([128, BS * 8 * S], BF16, tag="enctb", name="enctb")
        nc.sync.dma_start(
            enctb[:].rearrange("p (b k s) -> p b k s", b=BS, k=8),
            ENCT.ap().rearrange("b (k p) s -> p b k s", k=8))
        offsb = pf.tile([T, BS * S], F32, tag="offsb", name="offsb")
        nc.sync.dma_start(
            offsb[:].rearrange("p (b s) -> p b s", b=BS),
            OFFS.ap().rearrange("b t s -> t b s"))
        woutt = pf.tile([128, 12 * D], BF16, tag="woutt", name="woutt")
        nc.sync.dma_start(
            woutt[:].rearrange("p (k c) -> p k c", k=12),
            WOUTT.ap()[0:1536].rearrange("(k p) c -> p k c", k=12))
        woutb = pf.tile([1, D], BF16, tag="woutb", name="woutb")
        nc.sync.dma_start(woutb[:], WOUTT.ap()[1536:1537, :])

        st0 = linit(0)
        st1 = linit(1)
        # hT1 col 0 = initial c2_1
        nc.vector.tensor_copy(
            hT[1][:].rearrange("p (k b t) -> p k b t", k=4, b=BS)[:, :, :, 0],
            c2T0b[:, 128:256].rearrange("p (k r) -> p k r", k=4)[:, :, 0:BS])
        for ss in range(T + LAG + 1):
            if ss < T:
                lstep_mm(st0, ss)                # PE: inj + rounds L0(t)
            if LAG < ss <= T + LAG:
                lstep_transpose(st1, ss - LAG - 1)  # PE: T_c2 L1(t'-1)
            if ss < T:
                lstep_sigma(st0, ss)
            if LAG <= ss < T + LAG:
                lstep_mm(st1, ss - LAG)          # PE: bias/xpart/whh L1(t')
            if LAG <= ss < T + LAG:
                lstep_sigma(st1, ss - LAG)
            if ss < T:
                lstep_transpose(st0, ss)         # PE: T_c2 L0(t) (ready)
            if ss <= T:
                lstep_branch(st0)                # PE: T_h2 L0(t-1)

        # ---------------- deferred o-gate / h2_1 reconstruction ----------
        with tc.tile_pool(name="po", bufs=1) as po:
            for mc in range(2):
                msl = slice(128 * mc, 128 * (mc + 1))
                ps = gtile(mc, [128, 512])
                for k in range(4):
                    nc.tensor.matmul(
                        ps[:], hT[0][:, 256 * k + 128 * mc:
                                     256 * k + 128 * (mc + 1)],
                        wiho[:, 512 * k:512 * (k + 1)],
                        start=(k == 0), stop=False)
                for k in range(4):
                    nc.tensor.matmul(
                        ps[:], hT[1][:, 256 * k + 128 * mc:
                                     256 * k + 128 * (mc + 1)],
                        whho[:, 512 * k:512 * (k + 1)],
                        start=False, stop=False)
                nc.tensor.matmul(
                    ps[:], ones[0:1, 0:128], wihob[:],
                    start=False, stop=True)
                sob = po.tile([128, 512], BF16, tag=f"sob{mc}",
                              name=f"sob{mc}")
                nc.scalar.activation(sob[:], ps[:], AF.Sigmoid)
                for j in range(4):
                    tpo = psp.tile([128, 128], BF16, tag=f"tp{j % 2}",
                                   name=f"tp{j % 2}", bufs=2)
                    nc.tensor.transpose(
                        tpo[:], sob[:, 128 * j:128 * (j + 1)], identb[:])
                    dsl = slice(256 * j + 128 * mc, 256 * j + 128 * (mc + 1))
                    nc.vector.tensor_mul(
                        sTT[:, dsl], tpo[:], tanhH[:, dsl])

        # ---------------- Phase E: attention + out proj ----------------
        wkt2 = pa.tile([128, 4 * G], BF16, tag="wkt", name="wkt2")
        with tc.tile_pool(name="pe", bufs=1) as pe:
            sT = [sTT[:, 256 * k:256 * (k + 1)] for k in range(4)]

            xqT = []
            for m in range(8):
                ps = gtile(m % 2, [128, R])
                for k in range(4):
                    nc.tensor.matmul(
                        ps[:], wint[:, 1024 * k + 128 * m:1024 * k + 128 * (m + 1)],
                        sT[k], start=(k == 0), stop=(k == 3))
                xq = wkt2[:, 256 * m:256 * (m + 1)]
                if m % 2 == 0:
                    nc.scalar.copy(xq, ps[:])
                else:
                    nc.vector.tensor_copy(xq, ps[:])
                xqT.append(xq)

            ctxT = [wkt2[:, 2048 + 256 * m:2048 + 256 * (m + 1)]
                    for m in range(8)]
            for b in range(BS):
                bsl = slice(T * b, T * (b + 1))
                eps = psp.tile([T, S], F32, tag=f"g{b % 2}",
                               name=f"g{b % 2}", bufs=2)
                for k in range(8):
                    nc.tensor.matmul(
                        eps[:], xqT[k][:, bsl],
                        enctb[:, 1024 * b + 128 * k:1024 * b + 128 * (k + 1)],
                        start=(k == 0), stop=(k == 7))
                esb = pe.tile([T, S], F32, tag=f"esb{b % 2}",
                              name=f"esb{b % 2}")
                nc.vector.tensor_add(esb[:], eps[:],
                                     offsb[:, 128 * b:128 * (b + 1)])
                negmax = pe.tile([T, 1], F32, tag=f"negmax{b % 2}",
                                 name=f"negmax{b % 2}")
                nc.vector.reduce_max(
                    negmax[:], esb[:], axis=mybir.AxisListType.X, negate=True)
                expE = pe.tile([T, S], F32, tag=f"expE{b % 2}",
                               name=f"expE{b % 2}")
                den = pe.tile([T, 1], F32, tag=f"den{b % 2}",
                              name=f"den{b % 2}")
                nc.scalar.activation(
                    expE[:], esb[:], AF.Exp, bias=negmax[:], accum_out=den[:])
                rden = pe.tile([T, 1], F32, tag=f"rden{b % 2}",
                               name=f"rden{b % 2}")
                nc.vector.reciprocal(rden[:], den[:])
                attn = pe.tile([T, S], F32, tag=f"attn{b % 2}",
                               name=f"attn{b % 2}")
                nc.vector.tensor_scalar_mul(attn[:], expE[:], rden[:])
                tpa = psp.tile([S, T], F32, tag=f"tp{b % 2}",
                               name=f"tp{b % 2}", bufs=2)
                nc.tensor.transpose(tpa[:], attn[:], ident[0:T, 0:T])
                atsb = pe.tile([S, T], BF16, tag=f"atsb{b % 2}",
                               name=f"atsb{b % 2}")
                nc.vector.tensor_copy(atsb[:], tpa[:])
                for m in range(8):
                    psc = gtile(m % 2, [128, T])
                    nc.tensor.matmul(
                        psc[:], encb[:, 1024 * b + 128 * m:1024 * b + 128 * (m + 1)],
                        atsb[:], start=True, stop=True)
                    if m % 2 == 0:
                        nc.scalar.copy(ctxT[m][:, bsl], psc[:])
                    else:
                        nc.vector.tensor_copy(ctxT[m][:, bsl], psc[:])

            outflat = OUT.ap().rearrange("b t d -> (b t) d")
            lhs_all = ctxT + sT + [ones]
            wt_all = [woutt[:, 512 * k:512 * (k + 1)] for k in range(12)] \
                + [woutb[:]]
            for mc in range(2):
                msl = slice(128 * mc, 128 * (mc + 1))
                ps = gtile(mc, [128, D])
                for k in range(13):
                    nc.tensor.matmul(
                        ps[:], lhs_all[k][:, msl], wt_all[k],
                        start=(k == 0), stop=(k == 12))
                osb = pa.tile([128, D], F32, tag=f"stg{mc}", name=f"osb{mc}")
                nc.scalar.activation(osb[:], ps[:], AF.Tanh)
                nc.sync.dma_start(outflat[msl, :], osb[:])

    nc.compile()
    return nc


def assemble(results):
    full = np.concatenate([r["out"] for r in results], axis=0)  # [B, T, D]
    outs = full.transpose(1, 0, 2)                              # [T, B, D]
    return np.ascontiguousarray(outs.reshape(-1, D).reshape(-1, T, D))


_nc_cache = None


def kernel(**inputs):
    global _nc_cache
    in_maps = host_prep(inputs)
    if _nc_cache is None:
        _nc_cache = build_program()
    res = run_bass_kernel_spmd(_nc_cache, in_maps, list(range(NCORES)))
    return assemble(res.results)
